# revision 13
# baseline (speedup 1.0000x reference)
"""CFSDP (density-peaks clustering) on 8 Trainium2 NeuronCores.

Pipeline (N=8192 points, D=64, row-sharded 1024 rows/core):
  d2(i,j) = ||xi-xj||^2 via one K=66 augmented matmul per tile:
      u_i = (-2*x_i, sq_i, 1),  v_j = (x_j, 1, sq_j),  d2 = u_i . v_j
  All O(N^2) math runs on squared distances (sqrt is monotone, so order
  stats / argmin / percentile commute with it):
    L1: count(d2 < t_b) for 16 thresholds around the predicted 2%-quantile
        (ACT sigmoid step fn + accumulate) -> host interpolates dc^2.
    L2: rho_i = sum_j exp(-d2_ij/dc^2) (ACT Exp + accumulate, scale from SBUF).
    host: stable-sort rows by rho desc; "higher density" mask becomes a
        per-row prefix of the sorted column order.
    L3: delta_i^2 = min over prefix window of d2 (vector.tensor_mask_reduce,
        per-partition index window, on negated-d2 PSUM tiles).
  Host finishes: delta fallback (row max) for top-density rows, nhd argmin
  (lazy, only for non-center points), center ranks, label propagation scan.
"""

import os
import numpy as np

N = 8192
D = 64
NCORES = 8
ROWS = N // NCORES          # 1024 rows per core
P = 128                     # partitions
RB = ROWS // P              # 8 row-blocks per core
FD = 2048                   # free-dim group (4 PSUM banks)
G = N // FD                 # 4 col-groups per row
K = D + 4                   # 68 (augmented contraction dim, sq split hi+lo)
MM_N = 512                  # cols per matmul (one PSUM bank output)
MM_PER_G = FD // MM_N       # 4

NT = 4                      # percentile-count thresholds
L1_W = 1024                 # cols counted per threshold
DC2_CENTER = 86.2           # chi^2_64-predicted 2%-quantile of d2 (randn data)
DC2_GRID = (DC2_CENTER * (1.0 + (np.arange(NT) - (NT - 1) / 2) * 0.023)).astype(
    np.float64
)                           # +-3.5% bracket, 2.3% spacing
SIG_ALPHA = 2.0e4           # sigmoid step sharpness (soft window ~1e-3 in d2)
PCT = 2.0
FLT_MAX = float(np.finfo(np.float32).max)
PEN_BIG = 1e38              # penalty added beyond the prefix cutoff
PEN_ALPHA = 1e31            # relu penalty slope (ACT-built mask, L3)
WW = 1024                   # L3 boundary mask window width
NCOL = G + 1                # L3 output cols per block (G group slots + window)
EMPTY_SENTINEL = 1e37       # accum >= this => empty prefix window

# threshold b is counted on group (m, g) of every core (1/16 of the matrix
# per threshold => ~4.2M samples each; different rows+cols per threshold)
L1_GROUPS = [(b % RB, 1 + b % (G - 1)) for b in range(NT)]  # g>0: diag-free
DC2_STEP = float(DC2_CENTER * 0.023)
M_TOT = float(N) * float(N)
K_POS = PCT / 100.0 * (M_TOT - 1.0)
P_OFF = (K_POS - N) / (M_TOT - N)      # diag-free target CDF
CSTAR = float(P_OFF * P * L1_W)        # target count over the device sample

_programs: dict = {}

# ---------------------------------------------------------------------------
# Fast path: all-centers verification.
#
# For this problem's regime (randn data, thresholds 0.5) every point is a
# cluster center: rho_i >= 1 (diagonal exp(0) term) > rho_t, and
# delta_i >= min_{j!=i} dist_ij ~ 6.1 >> delta_t.  When that holds, labels
# are exactly arange(N) regardless of rho ordering.  One launch certifies it:
# compute the (upper-triangle) pairwise d2 and check no off-diagonal pair is
# within the threshold.  Groups of PSUM output are drained alternately by the
# ACT engine (sigmoid step counts, which also absorb the diagonal cells: each
# contributes ~1, totalling exactly 1024/core) and the DVE (min-reduce over
# diag-free groups).  Host passes iff rho_t < 1, every DVE min is above
# delta_t^2 + margin, and every ACT count matches its expected value.
# Otherwise we fall back to the full pipeline below.
#
# Per-core layout (core c, slot s = row-block r = 8s + c, 128 rows each):
#   U  [0:1024)      lhsT for the 8 slots
#   B  [1024:8192)   per-slot boundary cols  [128(r+1), +896) of V (padded
#                    with safely sub-diagonal cols where it runs off the end)
#   D  [8192:9216)   per-slot diagonal cols  [128r, +128) of V
#   VS [9216:16384)  shared V cols [1024:8192); slot s's tail reads
#                    VS[1024s : 7168)  (global cols [1024(s+1), 8192))
# Coverage of block r: [128r, 8192) with no gaps (tail start 1024(s+1)
# always <= boundary end 128(r+1)+896), so every upper pair is seen >= once;
# duplicates and sub-diagonal pad cells are true off-diagonal pairs and
# cannot fake a pass.
# ---------------------------------------------------------------------------

KV = 70                     # fp8 contraction: x(64) + sq hi/lo/lolo(3) + ones(3)
VER_ALPHA = 2.0             # sigmoid step sharpness (in d2 units)
VER_THETA = 20.0            # count threshold: flags any pair with d2 < ~20
VER_W = 896                 # boundary width: covers slot spread 7*128
VER_U0, VER_B0, VER_D0, VER_VS0 = 0, 1024, 8192, 9216
VER_COLS = 16384
VER_MARGIN = 14.0           # required gap above delta_t^2 (fp8 d2 err budget)
VER_T2MAX = 6.0             # max delta_t^2 the count certification covers


def _ver_groups():
    """Static PSUM-group schedule shared by the builder and the host check.

    Returns a list of dicts: tiles = [(slot, uv_col, width)], w = total
    width, eng = 'act' (sigmoid count) or 'dve' (min-reduce), diag flag.
    """
    groups = []
    for pair in range(4):
        tiles = []
        for s in (2 * pair, 2 * pair + 1):
            off = VER_B0 + VER_W * s
            tiles.append((s, off, 512))
            tiles.append((s, off + 512, 384))
        groups.append(dict(kind="b", tiles=tiles, w=1792, diag=False))
    groups.append(
        dict(
            kind="d",
            tiles=[(s, VER_D0 + 128 * s, 128) for s in range(8)],
            w=1024,
            diag=True,
        )
    )
    tail = []
    for s in range(6, -1, -1):  # descending s matches the VS DMA chunk order
        for k in range(14 - 2 * s):
            tail.append((s, VER_VS0 + 1024 * s + 512 * k, 512))
    for i in range(0, len(tail), 4):
        groups.append(dict(kind="t", tiles=tail[i:i + 4], w=2048, diag=False))
    # engine assignment: diag group must be ACT; balance projected busy time
    # (measured: ACT (w+352)/1.2 ns, DVE reduce ~0.92 elem/ns from PSUM)
    a_ns = d_ns = 0.0
    for g in groups:
        act_cost = (g["w"] + 352) / 1.2
        dve_cost = g["w"] / 0.92
        if g["diag"] or (not g["diag"] and a_ns + act_cost <= d_ns + dve_cost):
            g["eng"] = "act"
            a_ns += act_cost
        else:
            g["eng"] = "dve"
            d_ns += dve_cost
    return groups


def _build_ver():
    import concourse.mybir as mybir
    import concourse.tile as tile
    from concourse import bacc

    f32 = mybir.dt.float32
    f8 = mybir.dt.float8e4
    groups = _ver_groups()
    ng = len(groups)
    nc = bacc.Bacc("TRN2", debug=False, enable_asserts=False)
    uv_d = nc.dram_tensor("uv", [KV, VER_COLS], f8, kind="ExternalInput")
    out_d = nc.dram_tensor("vout", [P, ng], f32, kind="ExternalOutput")

    with tile.TileContext(nc) as tc:
        with (
            tc.tile_pool(name="inp", bufs=1) as inp,
            tc.tile_pool(name="stat", bufs=1) as stat,
            tc.tile_pool(name="trash", bufs=2) as trash_p,
            tc.tile_pool(name="psum", bufs=2, space="PSUM") as psum_p,
        ):
            uv_sb = inp.tile([KV, VER_COLS], f8)
            _pe_warmup(nc, tc, inp, psum_p, mybir)
            # DMA in consumption order, alternating the two free queues
            # (each DMA_DIRECT2D issue costs ~0.6-1us of queue time; the
            # stream rate ~90GB/s barely exceeds PE consumption in fp8, so
            # chunks release dependencies just ahead of the matmuls)
            chunks = [
                (0, 1920),                 # U + boundary slot 0
                (1920, 896),               # boundary slot 1
                (2816, 1792),              # boundary pair 1
                (4608, 1792),              # boundary pair 2
                (6400, 1792),              # boundary pair 3
                (VER_D0, 1024),            # diag cols
                (VER_VS0 + 5120, 2048),    # VS chunks, descending
                (VER_VS0 + 3072, 2048),
                (VER_VS0 + 1024, 2048),
                (VER_VS0, 1024),
            ]
            for i, (a, w) in enumerate(chunks):
                eng = nc.sync if i % 2 == 0 else nc.gpsimd
                eng.dma_start(out=uv_sb[:, a:a + w], in_=uv_d[:, a:a + w])

            outs = stat.tile([P, ng], f32)
            bias_sb = stat.tile([P, 1], f32)
            nc.vector.memset(bias_sb[:], float(VER_ALPHA * VER_THETA))
            warmact = stat.tile([P, 2], f32)
            nc.vector.memset(warmact[:, 0:1], 0.0)
            nc.scalar.activation(
                warmact[:, 1:2], warmact[:, 0:1],
                mybir.ActivationFunctionType.Sigmoid, bias=bias_sb[:, 0:1],
                scale=1.0,
            )
            for gi, g in enumerate(groups):
                psum = psum_p.tile([P, 2048], f32, tag="psum")
                col = 0
                for (s, off, wid) in g["tiles"]:
                    nc.tensor.matmul(
                        psum[:, col:col + wid],
                        uv_sb[:, 128 * s:128 * (s + 1)],
                        uv_sb[:, off:off + wid],
                        start=True,
                        stop=True,
                    )
                    col += wid
                if g["eng"] == "act":
                    t = trash_p.tile([P, 2048], f32, tag="trash")
                    nc.scalar.activation(
                        t[:, 0:g["w"]],
                        psum[:, 0:g["w"]],
                        mybir.ActivationFunctionType.Sigmoid,
                        bias=bias_sb[:, 0:1],
                        scale=float(-VER_ALPHA),
                        accum_out=outs[:, gi:gi + 1],
                    )
                else:
                    nc.vector.tensor_reduce(
                        outs[:, gi:gi + 1],
                        psum[:, 0:g["w"]],
                        axis=mybir.AxisListType.X,
                        op=mybir.AluOpType.min,
                    )
            nc.gpsimd.dma_start(out=out_d[:], in_=outs[:])
    nc.compile()
    return nc


def _augmented_f8(data):
    """fp8(e4m3) augmented operands for the verification GEMM (K=KV=70).

    u_i = (-2x_i, sqh_i, sql_i, sqll_i, 1, 1, 1)
    v_j = (x_j,   1,     1,     1,      sqh_j, sql_j, sqll_j)
    u.v = -2<x_i,x_j> + sq_i + sq_j = d2_ij, with |err| typically < ~5
    (dot-product quantization sigma ~1, sq-split err < 0.15).
    """
    import ml_dtypes

    f8 = ml_dtypes.float8_e4m3fn
    sq = np.einsum("ij,ij->i", data, data, dtype=np.float32)
    sh = sq.astype(f8)
    sl = (sq - sh.astype(np.float32)).astype(f8)
    sll = (sq - sh.astype(np.float32) - sl.astype(np.float32)).astype(f8)
    ones = np.ones((N, 1), f8)
    col = lambda a: a[:, None]
    U8 = np.concatenate(
        [(-2.0 * data).astype(f8), col(sh), col(sl), col(sll), ones, ones, ones],
        axis=1,
    )
    V8 = np.concatenate(
        [data.astype(f8), ones, ones, ones, col(sh), col(sl), col(sll)], axis=1
    )
    return U8, V8


def _ver_in_maps(U8, V8):
    """Per-core uv buffers for the verification launch."""
    VT = np.ascontiguousarray(V8.T)  # [KV, N]
    in_maps = []
    for c in range(NCORES):
        buf = np.empty((KV, VER_COLS), VT.dtype)
        for s in range(8):
            r = 8 * s + c
            buf[:, 128 * s:128 * (s + 1)] = U8[128 * r:128 * (r + 1)].T
            b0 = 128 * (r + 1)
            w = min(VER_W, N - b0)
            a = VER_B0 + VER_W * s
            if w > 0:
                buf[:, a:a + w] = VT[:, b0:b0 + w]
            if w < VER_W:  # pad with strictly sub-diagonal (safe) columns
                buf[:, a + w:a + VER_W] = VT[:, 0:VER_W - w]
            buf[:, VER_D0 + 128 * s:VER_D0 + 128 * (s + 1)] = VT[
                :, 128 * r:128 * (r + 1)
            ]
        buf[:, VER_VS0:VER_COLS] = VT[:, 1024:N]
        in_maps.append({"uv": buf})
    return in_maps


def _ver_decision(results, rho_t, delta_t):
    """True iff the device output certifies that every point is a center."""
    if not (rho_t < 1.0 - 1e-6):
        return False
    t2 = max(float(delta_t), 0.0) ** 2
    if t2 + VER_MARGIN > VER_T2MAX:
        return False
    groups = _ver_groups()
    for c in range(NCORES):
        out = results[c]["vout"]  # [P, ng]
        if not np.all(np.isfinite(out)):
            return False
        for gi, g in enumerate(groups):
            if g["eng"] == "act":
                cnt = float(out[:, gi].astype(np.float64).sum())
                expect = 1024.0 if g["diag"] else 0.0
                if abs(cnt - expect) > 0.4:
                    return False
            else:
                if float(out[:, gi].min()) <= t2 + VER_MARGIN:
                    return False
    return True


def _f32r(ap):
    import concourse.mybir as mybir

    return ap.bitcast(mybir.dt.float32r)



def _pe_warmup(nc, tc, inp, psum_p, mybir, n_mm=8):
    """Dense garbage-matmul burst at launch start: runs while the input DMA
    streams, trips the PE HAM un-throttle (~3.4us sustained busy) so real
    matmuls run at 2.4 GHz instead of 1.2 GHz."""
    f32 = mybir.dt.float32
    warm = inp.tile([K, P + MM_N], mybir.dt.bfloat16)
    nc.gpsimd.memset(warm[:], 1.0)
    wps = psum_p.tile([P, FD], f32, tag="psum")
    for j in range(n_mm):
        nc.tensor.matmul(
            wps[:, (j % MM_PER_G) * MM_N:((j % MM_PER_G) + 1) * MM_N],
            warm[:, :P],
            warm[:, P:P + MM_N],
            start=True,
            stop=True,
        )


def _build_l12():
    """Merged count + rho launch: dc^2 is computed ON DEVICE.

    Every core counts the SAME sample (rows 0..1023 via the shared `uvc`
    lhsT, diag-free col groups), so each core independently derives an
    identical dc^2 - no collectives. The CDF interpolation runs as tiny
    [1,8] vector ops; a PE ones-matmul does the partition reduction and a
    K=1 fp32 matmul broadcasts -1/dc^2 to all partitions for the rho phase.
    `dvec` carries host-computed control-variate corrections (in counts)
    that cancel the row/col sampling bias of the fixed sample.
    """
    import concourse.mybir as mybir
    import concourse.tile as tile
    from concourse import bacc

    f32 = mybir.dt.float32
    nc = bacc.Bacc("TRN2", debug=False, enable_asserts=False)
    bf16 = mybir.dt.bfloat16
    uv_d = nc.dram_tensor("uv", [K, ROWS + N], bf16, kind="ExternalInput")
    uvc_d = nc.dram_tensor("uvc", [K, ROWS], bf16, kind="ExternalInput")
    thr_d = nc.dram_tensor("thr", [P, NT], f32, kind="ExternalInput")
    tvec_d = nc.dram_tensor("tvec", [1, NT], f32, kind="ExternalInput")
    dvec_d = nc.dram_tensor("dvec", [1, NT], f32, kind="ExternalInput")
    cnt_d = nc.dram_tensor("counts", [P, NT], f32, kind="ExternalOutput")
    rho_d = nc.dram_tensor("rho", [P, RB], f32, kind="ExternalOutput")

    with tile.TileContext(nc) as tc:
        with (
            tc.tile_pool(name="inp", bufs=1) as inp,
            tc.tile_pool(name="stat", bufs=1) as stat,
            tc.tile_pool(name="trash", bufs=2) as trash_p,
            tc.tile_pool(name="psum", bufs=2, space="PSUM") as psum_p,
        ):
            uv_sb = inp.tile([K, ROWS + N], bf16)
            uvc_sb = inp.tile([K, ROWS], bf16)
            nc.sync.dma_start(out=uvc_sb[:], in_=uvc_d[:])
            for _g in (1, 2, 3):  # count-phase cols first; group 0 only for rho
                _a = ROWS + _g * FD
                nc.sync.dma_start(
                    out=uv_sb[:, _a:_a + FD], in_=uv_d[:, _a:_a + FD]
                )
            nc.sync.dma_start(out=uv_sb[:, 0:ROWS], in_=uv_d[:, 0:ROWS])
            nc.sync.dma_start(
                out=uv_sb[:, ROWS:ROWS + FD], in_=uv_d[:, ROWS:ROWS + FD]
            )
            thr_sb = inp.tile([P, NT], f32)
            nc.gpsimd.dma_start(out=thr_sb[:], in_=thr_d[:])
            tdv_sb = inp.tile([1, 2 * NT], f32)
            nc.gpsimd.dma_start(out=tdv_sb[:, 0:NT], in_=tvec_d[:])
            nc.gpsimd.dma_start(out=tdv_sb[:, NT:2 * NT], in_=dvec_d[:])
            cnts = stat.tile([P, NT], f32)
            warmact = stat.tile([P, 1], f32)
            nc.scalar.activation(
                warmact[:], thr_sb[:, 0:1],
                mybir.ActivationFunctionType.Sigmoid, bias=0.0, scale=1.0,
            )

            # ---- phase 1: counts over the shared sample -----------------
            for b, (m, g) in enumerate(L1_GROUPS):
                psum = psum_p.tile([P, FD], f32, tag="psum")
                for j in range(L1_W // MM_N):
                    nc.tensor.matmul(
                        psum[:, j * MM_N:(j + 1) * MM_N],
                        uvc_sb[:, m * P:(m + 1) * P],
                        uv_sb[:, ROWS + g * FD + j * MM_N: ROWS + g * FD + (j + 1) * MM_N],
                        start=True,
                        stop=True,
                    )
                t = trash_p.tile([P, L1_W], f32, tag="cntrash")
                nc.scalar.activation(
                    t[:],
                    psum[:, 0:L1_W],
                    mybir.ActivationFunctionType.Sigmoid,
                    bias=thr_sb[:, b:b + 1],
                    scale=float(-SIG_ALPHA),
                    accum_out=cnts[:, b:b + 1],
                )
            nc.gpsimd.dma_start(out=cnt_d[:], in_=cnts[:])

            # ---- phase 2: dc^2 from counts (identical on every core) ----
            ones_col = stat.tile([P, 1], f32)
            nc.vector.memset(ones_col[:], 1.0)
            ps_tot = psum_p.tile([1, NT], f32, tag="psum")
            nc.tensor.matmul(ps_tot[:], ones_col[:], cnts[:], start=True, stop=True)
            w = stat.tile([1, 8 * NT], f32)  # scratch lanes along free dim
            q = w[:, 0:NT]
            nc.vector.tensor_tensor(
                out=q, in0=ps_tot[:], in1=tdv_sb[:, NT:2 * NT],
                op=mybir.AluOpType.subtract,
            )
            NB_ = NT - 1
            a_ = w[:, NT:NT + NB_]
            nc.vector.tensor_scalar(
                out=a_, in0=q[:, 0:NB_], scalar1=CSTAR, scalar2=None,
                op0=mybir.AluOpType.is_le,
            )
            b_ = w[:, 2 * NT:2 * NT + NB_]
            nc.vector.tensor_scalar(
                out=b_, in0=q[:, 1:NT], scalar1=CSTAR, scalar2=None,
                op0=mybir.AluOpType.is_gt,
            )
            sel = w[:, 3 * NT:3 * NT + NB_]
            nc.vector.tensor_tensor(out=sel, in0=a_, in1=b_, op=mybir.AluOpType.mult)
            den = w[:, 4 * NT:4 * NT + NB_]
            nc.vector.tensor_tensor(
                out=den, in0=q[:, 1:NT], in1=q[:, 0:NB_],
                op=mybir.AluOpType.subtract,
            )
            rec = w[:, 5 * NT:5 * NT + NB_]
            nc.vector.reciprocal(rec, den)
            num = w[:, 6 * NT:6 * NT + NB_]
            nc.vector.tensor_scalar(
                out=num, in0=q[:, 0:NB_], scalar1=-1.0, scalar2=CSTAR,
                op0=mybir.AluOpType.mult, op1=mybir.AluOpType.add,
            )
            fr = w[:, 7 * NT:7 * NT + NB_]
            nc.vector.tensor_tensor(out=fr, in0=num, in1=rec, op=mybir.AluOpType.mult)
            nc.vector.tensor_scalar(
                out=fr, in0=fr, scalar1=float(DC2_STEP), scalar2=None,
                op0=mybir.AluOpType.mult,
            )
            nc.vector.tensor_tensor(
                out=fr, in0=fr, in1=tdv_sb[:, 0:NB_], op=mybir.AluOpType.add
            )
            nc.vector.tensor_tensor(out=fr, in0=fr, in1=sel, op=mybir.AluOpType.mult)
            sc = stat.tile([1, 4], f32)
            nc.vector.tensor_reduce(
                sc[:, 0:1], fr[:], axis=mybir.AxisListType.X, op=mybir.AluOpType.add
            )
            nc.vector.tensor_reduce(
                sc[:, 1:2], sel[:], axis=mybir.AxisListType.X, op=mybir.AluOpType.add
            )
            # guard: if no bracket, fall back to the grid center
            nc.vector.tensor_scalar(
                out=sc[:, 2:3], in0=sc[:, 1:2], scalar1=float(-DC2_CENTER),
                scalar2=float(DC2_CENTER), op0=mybir.AluOpType.mult,
                op1=mybir.AluOpType.add,
            )
            nc.vector.tensor_tensor(
                out=sc[:, 0:1], in0=sc[:, 0:1], in1=sc[:, 2:3],
                op=mybir.AluOpType.add,
            )
            nc.vector.reciprocal(sc[:, 3:4], sc[:, 0:1])
            nc.vector.tensor_scalar(
                out=sc[:, 3:4], in0=sc[:, 3:4], scalar1=-1.0, scalar2=None,
                op0=mybir.AluOpType.mult,
            )
            ones_row = stat.tile([1, P], f32)
            nc.vector.memset(ones_row[:], 1.0)
            ps_b = psum_p.tile([P, 1], f32, tag="psum")
            nc.tensor.matmul(ps_b[:], ones_row[:], sc[:, 3:4], start=True, stop=True)
            scl_sb = stat.tile([P, 1], f32)
            nc.vector.tensor_copy(scl_sb[:], ps_b[:])

            # ---- phase 3: rho ------------------------------------------
            parts = stat.tile([P, RB * G], f32)
            rho_sb = stat.tile([P, RB], f32)
            for m in range(RB):
                for g in range(G):
                    psum = psum_p.tile([P, FD], f32, tag="psum")
                    for j in range(MM_PER_G):
                        nc.tensor.matmul(
                            psum[:, j * MM_N:(j + 1) * MM_N],
                            uv_sb[:, m * P:(m + 1) * P],
                            uv_sb[:, ROWS + g * FD + j * MM_N: ROWS + g * FD + (j + 1) * MM_N],
                            start=True,
                            stop=True,
                        )
                    t = trash_p.tile([P, FD], f32, tag="trash")
                    q2 = m * G + g
                    nc.scalar.activation(
                        t[:],
                        psum[:],
                        mybir.ActivationFunctionType.Exp,
                        bias=0.0,
                        scale=scl_sb[:, 0:1],
                        accum_out=parts[:, q2:q2 + 1],
                    )
                nc.vector.tensor_reduce(
                    rho_sb[:, m:m + 1],
                    parts[:, m * G:(m + 1) * G],
                    axis=mybir.AxisListType.X,
                    op=mybir.AluOpType.add,
                )
            nc.sync.dma_start(out=rho_d[:], in_=rho_sb[:])
    nc.compile()
    return nc


def _build_l3():
    """Delta pass on rho-sorted data (round-robin block interleaving).

    Core c holds sorted row-blocks b = 8m + c (m = 0..7). For local block m:
      boundary col-group g_b = m//2, window base w_lo = 1024*(m%2)
      (cutoffs of every core's block-m rows lie in [w_lo, w_lo+1024) of
      group g_b, ties aside - those are patched on host).
    Structure per block:
      groups g < g_b:                plain min-reduce of the whole group
      boundary prefix [0, w_lo):     plain min-reduce (odd m only)
      boundary window [w_lo,+1024):  penalty mask (iota >= cutrel)*BIG, add,
                                     min-reduce
      columns beyond w_lo+1024 and groups g > g_b: skipped entirely.
    """
    import concourse.mybir as mybir
    import concourse.tile as tile
    from concourse import bacc

    f32 = mybir.dt.float32
    nc = bacc.Bacc("TRN2", debug=False, enable_asserts=False)
    bf16 = mybir.dt.bfloat16
    uv_d = nc.dram_tensor("uv", [K, ROWS + N], bf16, kind="ExternalInput")
    cut_d = nc.dram_tensor("cut", [P, RB], f32, kind="ExternalInput")
    iota_d = nc.dram_tensor("iota", [P, WW], f32, kind="ExternalInput")
    dmin_d = nc.dram_tensor("dmin", [P, RB * NCOL], f32, kind="ExternalOutput")

    with tile.TileContext(nc) as tc:
        with (
            tc.tile_pool(name="inp", bufs=1) as inp,
            tc.tile_pool(name="stat", bufs=1) as stat,
            tc.tile_pool(name="trash", bufs=3) as trash_p,
            tc.tile_pool(name="pen", bufs=3) as pen_p,
            tc.tile_pool(name="psum", bufs=2, space="PSUM") as psum_p,
        ):
            uv_sb = inp.tile([K, ROWS + N], bf16)
            nc.sync.dma_start(out=uv_sb[:, 0:ROWS], in_=uv_d[:, 0:ROWS])
            for _g in range(G):
                _a = ROWS + _g * FD
                nc.sync.dma_start(
                    out=uv_sb[:, _a:_a + FD], in_=uv_d[:, _a:_a + FD]
                )
            cut_sb = inp.tile([P, RB], f32)
            nc.gpsimd.dma_start(out=cut_sb[:], in_=cut_d[:])
            iota_sb = inp.tile([P, WW], f32)
            nc.gpsimd.dma_start(out=iota_sb[:], in_=iota_d[:])
            dmin_sb = stat.tile([P, RB * NCOL], f32)

            for m in range(RB):
                gb = m // 2
                w_lo = WW * (m % 2)
                pen = pen_p.tile([P, WW], f32, tag="pen")
                # cutrel (host-clamped to [0, WW]) is relative to w_lo
                nc.vector.tensor_scalar(
                    out=pen[:],
                    in0=iota_sb[:],
                    scalar1=cut_sb[:, m:m + 1],
                    scalar2=PEN_BIG,
                    op0=mybir.AluOpType.is_ge,
                    op1=mybir.AluOpType.mult,
                )
                for g in range(gb + 1):
                    ncols = FD if g < gb else w_lo + WW
                    psum = psum_p.tile([P, FD], f32, tag="psum")
                    for j in range(ncols // MM_N):
                        nc.tensor.matmul(
                            psum[:, j * MM_N:(j + 1) * MM_N],
                            uv_sb[:, m * P:(m + 1) * P],
                            uv_sb[:, ROWS + g * FD + j * MM_N: ROWS + g * FD + (j + 1) * MM_N],
                            start=True,
                            stop=True,
                        )
                    q = m * NCOL + g
                    if g < gb:
                        nc.vector.tensor_reduce(
                            dmin_sb[:, q:q + 1],
                            psum[:],
                            axis=mybir.AxisListType.X,
                            op=mybir.AluOpType.min,
                        )
                    else:
                        if w_lo > 0:
                            nc.vector.tensor_reduce(
                                dmin_sb[:, q:q + 1],
                                psum[:, 0:w_lo],
                                axis=mybir.AxisListType.X,
                                op=mybir.AluOpType.min,
                            )
                        t = trash_p.tile([P, WW], f32, tag="trash")
                        nc.vector.tensor_tensor(
                            out=t[:],
                            in0=psum[:, w_lo:w_lo + WW],
                            in1=pen[:],
                            op=mybir.AluOpType.add,
                        )
                        nc.vector.tensor_reduce(
                            dmin_sb[:, m * NCOL + G:m * NCOL + G + 1],
                            t[:],
                            axis=mybir.AxisListType.X,
                            op=mybir.AluOpType.min,
                        )
            nc.gpsimd.dma_start(out=dmin_d[:], in_=dmin_sb[:])
    nc.compile()
    return nc


_BUILDERS = {"l12": _build_l12, "l3": _build_l3, "ver": _build_ver}


def _get_program(name):
    if name not in _programs:
        _programs[name] = _BUILDERS[name]()
    return _programs[name]


TIMINGS = []  # (name, exec_time_ns) per launch, appended by _run


def _run(name, in_maps, trace=None):
    from concourse.bass_utils import run_bass_kernel_spmd

    if trace is None:
        trace = bool(int(os.environ.get("KERNEL_TRACE", "0")))
    nc = _get_program(name)
    res = run_bass_kernel_spmd(
        nc, in_maps, core_ids=list(range(NCORES)), trace=trace
    )
    TIMINGS.append((name, res.exec_time_ns))
    return res


def _augmented(data):
    """U (lhs rows) and V (rhs cols) of the K=68 augmented distance GEMM.

    bf16 operands with sq split into a bf16 hi+lo pair: d2 error ~0.04 abs
    (~5e-4 relative at the dc^2 scale), far inside every decision margin.
    """
    import ml_dtypes

    bf = ml_dtypes.bfloat16
    sq = np.einsum("ij,ij->i", data, data, dtype=np.float32).astype(np.float32)
    sqh = sq.astype(bf)
    sql = (sq - sqh.astype(np.float32)).astype(bf)
    ones = np.ones((N, 1), bf)
    zcol = lambda a: a[:, None]
    U = np.concatenate(
        [(-2.0 * data).astype(bf), zcol(sqh), zcol(sql), ones, ones], axis=1
    )
    V = np.concatenate(
        [data.astype(bf), ones, ones, zcol(sqh), zcol(sql)], axis=1
    )
    return U, V, sq


def _erf(x):
    """Abramowitz-Stegun 7.1.26 vectorized erf (|err| < 1.5e-7)."""
    s = np.sign(x)
    x = np.abs(x)
    t = 1.0 / (1.0 + 0.3275911 * x)
    y = 1.0 - (
        ((((1.061405429 * t - 1.453152027) * t) + 1.421413741) * t - 0.284496736)
        * t
        + 0.254829592
    ) * t * np.exp(-x * x)
    return s * y


def _phi(z):
    return 0.5 * (1.0 + _erf(z / np.sqrt(2.0)))


NGRID = 256


def _cv_corrections(sq):
    """Control-variate count corrections for the fixed count sample.

    Model P(d2 < t | sq_i, sq_j) ~ Phi((t - sq_i - sq_j)/(2 sqrt(sq_i sq_j/D)))
    and subtract the predicted row/col selection bias of the sampled
    rows/cols relative to the full point set.
    """
    sq64 = sq.astype(np.float64)
    step = N // NGRID
    grid = np.sort(sq64)[step // 2::step][:NGRID]

    def h(t, svals):
        s = svals[:, None]
        sp = grid[None, :]
        z = (t - s - sp) / (2.0 * np.sqrt(np.maximum(s * sp, 1e-9) / D))
        return _phi(z).mean(axis=1)

    dvec = np.zeros(NT)
    for b, (m, g) in enumerate(L1_GROUPS):
        t = float(DC2_GRID[b])
        h_all = h(t, grid).mean()
        d_row = h(t, sq64[m * P:(m + 1) * P]).mean() - h_all
        d_col = h(t, sq64[g * FD:g * FD + L1_W]).mean() - h_all
        dvec[b] = (d_row + d_col) * (P * L1_W)
    return dvec.astype(np.float32).reshape(1, NT)


def _interp_dc2(counts_by_core):
    """counts_by_core: list of [P, NT] arrays -> dc^2 via CDF interpolation."""
    M = float(N) * float(N)
    k_pos = PCT / 100.0 * (M - 1.0)
    p_off = (k_pos - N) / (M - N)  # diag cells (d2=0) all fall below any t_b

    tot = np.zeros(NT, np.float64)
    denom = np.zeros(NT, np.float64)
    for c in range(NCORES):
        cc = counts_by_core[c].astype(np.float64).sum(axis=0)  # [NT]
        for b, (m, g) in enumerate(L1_GROUPS):
            row0 = c * ROWS + m * P
            off = row0 - g * FD
            has_diag = 0 <= off <= L1_W - P
            tot[b] += cc[b] - (P if has_diag else 0)
            denom[b] += P * L1_W - (P if has_diag else 0)
    p_hat = tot / denom
    # p_hat should be increasing in b; enforce monotonicity for safety
    p_mono = np.maximum.accumulate(p_hat)
    if not (p_mono[0] <= p_off <= p_mono[-1]):
        return None  # bracket miss -> caller falls back to exact host path
    b_hi = int(np.searchsorted(p_mono, p_off, side="left"))
    if b_hi == 0:
        return float(DC2_GRID[0])
    b_lo = b_hi - 1
    p_lo, p_hi_v = p_mono[b_lo], p_mono[b_hi]
    frac = 0.0 if p_hi_v <= p_lo else (p_off - p_lo) / (p_hi_v - p_lo)
    return float(DC2_GRID[b_lo] + frac * (DC2_GRID[b_hi] - DC2_GRID[b_lo]))


def _host_fallback(data, rho_t, delta_t):
    """Pure-numpy reference path (only used if device assumptions break)."""
    data = np.asarray(data, np.float32)
    sq = np.sum(data * data, axis=1)
    d2 = sq[:, None] + sq[None, :] - 2.0 * (data @ data.T)
    dist = np.sqrt(np.maximum(d2, 0.0), dtype=np.float32)
    dc = np.percentile(dist, PCT)
    rho = np.exp(-((dist / dc) ** 2)).sum(axis=1).astype(np.float32)
    higher = rho[None, :] > rho[:, None]
    masked = np.where(higher, dist, np.inf)
    delta_m = masked.min(axis=1)
    nhd_m = masked.argmin(axis=1)
    has = higher.any(axis=1)
    delta = np.where(has, delta_m, dist.max(axis=1))
    nhd = np.where(has, nhd_m, np.arange(N))
    return _finish_labels(rho, delta, nhd, rho_t, delta_t)


def _finish_labels(rho, delta, nhd, rho_t, delta_t):
    is_center = (rho > rho_t) & (delta > delta_t)
    center_rank = np.cumsum(is_center.astype(np.int32)) - 1
    labels = np.where(is_center, center_rank, -1).astype(np.int32)
    order = np.argsort(-rho, kind="stable")
    for i in order:
        if labels[i] < 0:
            labels[i] = labels[nhd[i]]
    return labels


def kernel(data, rho_threshold, delta_threshold):
    data = np.ascontiguousarray(np.asarray(data, dtype=np.float32))
    assert data.shape == (N, D)
    rho_t = float(np.asarray(rho_threshold))
    delta_t = float(np.asarray(delta_threshold))

    # ---- fast path: certify "every point is a center" ------------------
    U8, V8 = _augmented_f8(data)
    rver = _run("ver", _ver_in_maps(U8, V8))
    if _ver_decision(rver.results, rho_t, delta_t):
        return np.arange(N, dtype=np.int32)

    U, V, sq = _augmented(data)
    VT = V.T  # [K, N]

    # ---- L12: counts -> on-device dc^2 -> rho (single launch) ----------
    thr = np.broadcast_to(
        (SIG_ALPHA * DC2_GRID).astype(np.float32)[None, :], (P, NT)
    ).copy()
    tvec = DC2_GRID.astype(np.float32).reshape(1, NT)
    dvec = _cv_corrections(sq)
    uvc = np.ascontiguousarray(np.concatenate([U[0:ROWS].T, VT], axis=1)[:, 0:ROWS])
    in_maps = [
        {
            "uv": np.ascontiguousarray(
                np.concatenate([U[c * ROWS:(c + 1) * ROWS].T, VT], axis=1)
            ),
            "uvc": uvc,
            "thr": thr,
            "tvec": tvec,
            "dvec": dvec,
        }
        for c in range(NCORES)
    ]
    r12 = _run("l12", in_maps)

    # validate the on-device dc interpolation from the counts output
    q = r12.results[0]["counts"].astype(np.float64).sum(axis=0) - dvec[0].astype(
        np.float64
    )
    brackets = [
        b for b in range(NT - 1) if q[b] <= CSTAR < q[b + 1]
    ]
    if len(brackets) != 1 or not np.all(np.diff(q) > 0):
        return _host_fallback(data, rho_t, delta_t)

    rho = np.empty(N, np.float32)
    for c in range(NCORES):
        out = r12.results[c]["rho"]  # [P, RB]
        rho[c * ROWS:(c + 1) * ROWS] = out.T.reshape(-1)
    if not np.all(np.isfinite(rho)) or rho.min() < 0.5 or rho.max() > N + 1:
        return _host_fallback(data, rho_t, delta_t)

    # ---- host: sort by rho desc; prefix cutoffs ------------------------
    order = np.argsort(-rho, kind="stable")
    rho_sorted = rho[order]
    # c_i = #points with rho strictly greater (ties excluded)
    cuts = np.searchsorted(-rho_sorted, -rho_sorted, side="left").astype(np.int64)

    data_p = data[order]
    sq_p = sq[order]
    Up = U[order]
    Vp = V[order]
    rhs_p = np.ascontiguousarray(Vp.T)

    # round-robin block interleave: core c <- sorted blocks 8m + c
    NB = N // P  # 64 sorted row-blocks
    blk_rows = np.arange(N).reshape(NB, P)
    core_rows = [blk_rows[np.arange(RB) * NCORES + c].reshape(-1) for c in range(NCORES)]

    iota_in = np.broadcast_to(
        np.arange(WW, dtype=np.float32)[None, :], (P, WW)
    ).copy()
    in_maps = []
    for c in range(NCORES):
        rows = core_rows[c]
        cutrel = np.empty((P, RB), np.float32)
        for m in range(RB):
            base = (m // 2) * FD + WW * (m % 2)
            cutrel[:, m] = np.clip(cuts[rows[m * P:(m + 1) * P]] - base, 0, WW)
        in_maps.append(
            {
                "uv": np.ascontiguousarray(
                    np.concatenate([Up[rows].T, rhs_p], axis=1)
                ),
                "cut": cutrel,
                "iota": iota_in,
            }
        )
    r3 = _run("l3", in_maps)
    # dmin[i] holds per-source minima; dcol[k] = (col_base, col_len) of source k
    dmin = np.full((N, NCOL), np.inf, np.float32)
    for c in range(NCORES):
        out = r3.results[c]["dmin"]  # [P, RB*NCOL]
        rows = core_rows[c]
        for m in range(RB):
            gb = m // 2
            w_lo = WW * (m % 2)
            blk = rows[m * P:(m + 1) * P]
            for g in range(gb):
                dmin[blk, g] = out[:, m * NCOL + g]
            if w_lo > 0:
                dmin[blk, gb] = out[:, m * NCOL + gb]
            dmin[blk, G] = out[:, m * NCOL + G]

    # ---- host: delta, fallback rows, centers, nhd (lazy), labels -------
    delta2_sorted = dmin.min(axis=1)

    # rho-tie rows whose cutoff dips below their block's boundary group: the
    # device's full-group reduce included a few extra columns; fix exactly.
    win_base = ((np.arange(N) // P) // NCORES) * WW  # 1024*m per sorted row
    straddle_fix = {}
    for i in np.nonzero(cuts < win_base)[0]:
        cut = int(cuts[i])
        if cut == 0:
            delta2_sorted[i] = np.inf
            continue
        d2row = sq_p[i] + sq_p[:cut] - 2.0 * (data_p[:cut] @ data_p[i])
        j = int(np.argmin(d2row))
        delta2_sorted[i] = d2row[j]
        straddle_fix[i] = j

    empty = delta2_sorted >= EMPTY_SENTINEL  # no higher-density point
    delta_sorted = np.sqrt(np.maximum(delta2_sorted, 0.0), dtype=np.float32)
    for i in np.nonzero(empty)[0]:
        d2row = sq_p[i] + sq_p - 2.0 * (data_p @ data_p[i])
        delta_sorted[i] = np.sqrt(max(float(np.max(np.maximum(d2row, 0.0))), 0.0))

    delta = np.empty(N, np.float32)
    delta[order] = delta_sorted

    is_center = (rho > rho_t) & (delta > delta_t)
    center_rank = np.cumsum(is_center.astype(np.int32)) - 1
    labels = np.where(is_center, center_rank, -1).astype(np.int32)

    need_nhd = ~is_center[order]  # sorted positions whose label must propagate
    nhd = np.arange(N, dtype=np.int64)  # default: self (matches reference)
    for i in np.nonzero(need_nhd)[0]:
        if empty[i]:
            continue  # nhd stays self, as in reference
        if i in straddle_fix:
            nhd[order[i]] = order[straddle_fix[i]]
            continue
        k = int(np.argmin(dmin[i]))
        m = (i // P) // NCORES
        gb = m // 2
        w_lo = WW * (m % 2)
        if k == G:
            c0, clen = gb * FD + w_lo, WW
        elif k == gb:
            c0, clen = gb * FD, w_lo
        else:
            c0, clen = k * FD, FD
        end_local = int(np.clip(cuts[i] - c0, 0, clen))
        cols = slice(c0, c0 + end_local)
        d2part = sq_p[i] + sq_p[cols] - 2.0 * (data_p[cols] @ data_p[i])
        j_local = int(np.argmin(d2part))
        nhd[order[i]] = order[c0 + j_local]

    for i in order:
        if labels[i] < 0:
            labels[i] = labels[nhd[i]]
    return labels.astype(np.int32)



# revision 14
# speedup vs baseline: 1.0011x; 1.0011x over previous
"""CFSDP (density-peaks clustering) on 8 Trainium2 NeuronCores.

Pipeline (N=8192 points, D=64, row-sharded 1024 rows/core):
  d2(i,j) = ||xi-xj||^2 via one K=66 augmented matmul per tile:
      u_i = (-2*x_i, sq_i, 1),  v_j = (x_j, 1, sq_j),  d2 = u_i . v_j
  All O(N^2) math runs on squared distances (sqrt is monotone, so order
  stats / argmin / percentile commute with it):
    L1: count(d2 < t_b) for 16 thresholds around the predicted 2%-quantile
        (ACT sigmoid step fn + accumulate) -> host interpolates dc^2.
    L2: rho_i = sum_j exp(-d2_ij/dc^2) (ACT Exp + accumulate, scale from SBUF).
    host: stable-sort rows by rho desc; "higher density" mask becomes a
        per-row prefix of the sorted column order.
    L3: delta_i^2 = min over prefix window of d2 (vector.tensor_mask_reduce,
        per-partition index window, on negated-d2 PSUM tiles).
  Host finishes: delta fallback (row max) for top-density rows, nhd argmin
  (lazy, only for non-center points), center ranks, label propagation scan.
"""

import os
import numpy as np

N = 8192
D = 64
NCORES = 8
ROWS = N // NCORES          # 1024 rows per core
P = 128                     # partitions
RB = ROWS // P              # 8 row-blocks per core
FD = 2048                   # free-dim group (4 PSUM banks)
G = N // FD                 # 4 col-groups per row
K = D + 4                   # 68 (augmented contraction dim, sq split hi+lo)
MM_N = 512                  # cols per matmul (one PSUM bank output)
MM_PER_G = FD // MM_N       # 4

NT = 4                      # percentile-count thresholds
L1_W = 1024                 # cols counted per threshold
DC2_CENTER = 86.2           # chi^2_64-predicted 2%-quantile of d2 (randn data)
DC2_GRID = (DC2_CENTER * (1.0 + (np.arange(NT) - (NT - 1) / 2) * 0.023)).astype(
    np.float64
)                           # +-3.5% bracket, 2.3% spacing
SIG_ALPHA = 2.0e4           # sigmoid step sharpness (soft window ~1e-3 in d2)
PCT = 2.0
FLT_MAX = float(np.finfo(np.float32).max)
PEN_BIG = 1e38              # penalty added beyond the prefix cutoff
PEN_ALPHA = 1e31            # relu penalty slope (ACT-built mask, L3)
WW = 1024                   # L3 boundary mask window width
NCOL = G + 1                # L3 output cols per block (G group slots + window)
EMPTY_SENTINEL = 1e37       # accum >= this => empty prefix window

# threshold b is counted on group (m, g) of every core (1/16 of the matrix
# per threshold => ~4.2M samples each; different rows+cols per threshold)
L1_GROUPS = [(b % RB, 1 + b % (G - 1)) for b in range(NT)]  # g>0: diag-free
DC2_STEP = float(DC2_CENTER * 0.023)
M_TOT = float(N) * float(N)
K_POS = PCT / 100.0 * (M_TOT - 1.0)
P_OFF = (K_POS - N) / (M_TOT - N)      # diag-free target CDF
CSTAR = float(P_OFF * P * L1_W)        # target count over the device sample

_programs: dict = {}

# ---------------------------------------------------------------------------
# Fast path: all-centers verification.
#
# For this problem's regime (randn data, thresholds 0.5) every point is a
# cluster center: rho_i >= 1 (diagonal exp(0) term) > rho_t, and
# delta_i >= min_{j!=i} dist_ij ~ 6.1 >> delta_t.  When that holds, labels
# are exactly arange(N) regardless of rho ordering.  One launch certifies it:
# compute the (upper-triangle) pairwise d2 and check no off-diagonal pair is
# within the threshold.  Groups of PSUM output are drained alternately by the
# ACT engine (sigmoid step counts, which also absorb the diagonal cells: each
# contributes ~1, totalling exactly 1024/core) and the DVE (min-reduce over
# diag-free groups).  Host passes iff rho_t < 1, every DVE min is above
# delta_t^2 + margin, and every ACT count matches its expected value.
# Otherwise we fall back to the full pipeline below.
#
# Per-core layout (core c, slot s = row-block r = 8s + c, 128 rows each):
#   U  [0:1024)      lhsT for the 8 slots
#   B  [1024:8192)   per-slot boundary cols  [128(r+1), +896) of V (padded
#                    with safely sub-diagonal cols where it runs off the end)
#   D  [8192:9216)   per-slot diagonal cols  [128r, +128) of V
#   VS [9216:16384)  shared V cols [1024:8192); slot s's tail reads
#                    VS[1024s : 7168)  (global cols [1024(s+1), 8192))
# Coverage of block r: [128r, 8192) with no gaps (tail start 1024(s+1)
# always <= boundary end 128(r+1)+896), so every upper pair is seen >= once;
# duplicates and sub-diagonal pad cells are true off-diagonal pairs and
# cannot fake a pass.
# ---------------------------------------------------------------------------

KV = 70                     # fp8 contraction: x(64) + sq hi/lo/lolo(3) + ones(3)
VER_ALPHA = 2.0             # sigmoid step sharpness (in d2 units)
VER_THETA = 20.0            # count threshold: flags any pair with d2 < ~20
VER_W = 896                 # boundary width: covers slot spread 7*128
VER_U0, VER_B0, VER_D0, VER_VS0 = 0, 1024, 8192, 9216
VER_COLS = 16384
VER_MARGIN = 14.0           # required gap above delta_t^2 (fp8 d2 err budget)
VER_T2MAX = 6.0             # max delta_t^2 the count certification covers


def _ver_groups():
    """Static PSUM-group schedule shared by the builder and the host check.

    Returns a list of dicts: tiles = [(slot, uv_col, width)], w = total
    width, eng = 'act' (sigmoid count) or 'dve' (min-reduce), diag flag.
    """
    groups = []
    for pair in range(4):
        tiles = []
        for s in (2 * pair, 2 * pair + 1):
            off = VER_B0 + VER_W * s
            tiles.append((s, off, 512))
            tiles.append((s, off + 512, 384))
        groups.append(dict(kind="b", tiles=tiles, w=1792, diag=False))
    groups.append(
        dict(
            kind="d",
            tiles=[(s, VER_D0 + 128 * s, 128) for s in range(8)],
            w=1024,
            diag=True,
        )
    )
    tail = []
    for s in range(6, -1, -1):  # descending s matches the VS DMA chunk order
        for k in range(14 - 2 * s):
            tail.append((s, VER_VS0 + 1024 * s + 512 * k, 512))
    for i in range(0, len(tail), 4):
        groups.append(dict(kind="t", tiles=tail[i:i + 4], w=2048, diag=False))
    # engine assignment: diag group must be ACT; balance projected busy time
    # (measured: ACT (w+352)/1.2 ns, DVE reduce ~0.92 elem/ns from PSUM)
    a_ns = d_ns = 0.0
    for g in groups:
        act_cost = (g["w"] + 352) / 1.2
        dve_cost = g["w"] / 0.92
        if g["diag"] or (not g["diag"] and a_ns + act_cost <= d_ns + dve_cost):
            g["eng"] = "act"
            a_ns += act_cost
        else:
            g["eng"] = "dve"
            d_ns += dve_cost
    return groups


def _build_ver():
    import concourse.mybir as mybir
    import concourse.tile as tile
    from concourse import bacc

    f32 = mybir.dt.float32
    f8 = mybir.dt.float8e4
    groups = _ver_groups()
    ng = len(groups)
    nc = bacc.Bacc("TRN2", debug=False, enable_asserts=False)
    uv_d = nc.dram_tensor("uv", [KV, VER_COLS], f8, kind="ExternalInput")
    out_d = nc.dram_tensor("vout", [P, ng], f32, kind="ExternalOutput")

    with tile.TileContext(nc) as tc:
        with (
            tc.tile_pool(name="inp", bufs=1) as inp,
            tc.tile_pool(name="stat", bufs=1) as stat,
            tc.tile_pool(name="trash", bufs=2) as trash_p,
            tc.tile_pool(name="psum", bufs=2, space="PSUM") as psum_p,
        ):
            uv_sb = inp.tile([KV, VER_COLS], f8)
            _pe_warmup(nc, tc, inp, psum_p, mybir)
            # DMA in consumption order, alternating the two free queues
            # (each DMA_DIRECT2D issue costs ~0.6-1us of queue time; the
            # stream rate ~90GB/s barely exceeds PE consumption in fp8, so
            # chunks release dependencies just ahead of the matmuls)
            chunks = [
                (0, 1920),                 # U + boundary slot 0
                (1920, 896),               # boundary slot 1
                (2816, 1792),              # boundary pair 1
                (4608, 1792),              # boundary pair 2
                (6400, 1792),              # boundary pair 3
                (VER_D0, 1024),            # diag cols
                (VER_VS0 + 5120, 2048),    # VS chunks, descending
                (VER_VS0 + 3072, 2048),
                (VER_VS0 + 1024, 2048),
                (VER_VS0, 1024),
            ]
            for i, (a, w) in enumerate(chunks):
                eng = nc.sync if i % 2 == 0 else nc.gpsimd
                eng.dma_start(out=uv_sb[:, a:a + w], in_=uv_d[:, a:a + w])

            outs = stat.tile([P, ng], f32)
            bias_sb = stat.tile([P, 1], f32)
            nc.vector.memset(bias_sb[:], float(VER_ALPHA * VER_THETA))
            warmact = stat.tile([P, 2], f32)
            nc.vector.memset(warmact[:, 0:1], 0.0)
            nc.scalar.activation(
                warmact[:, 1:2], warmact[:, 0:1],
                mybir.ActivationFunctionType.Sigmoid, bias=bias_sb[:, 0:1],
                scale=1.0,
            )
            for gi, g in enumerate(groups):
                psum = psum_p.tile([P, 2048], f32, tag="psum")
                col = 0
                for (s, off, wid) in g["tiles"]:
                    nc.tensor.matmul(
                        psum[:, col:col + wid],
                        uv_sb[:, 128 * s:128 * (s + 1)],
                        uv_sb[:, off:off + wid],
                        start=True,
                        stop=True,
                    )
                    col += wid
                if g["eng"] == "act":
                    t = trash_p.tile([P, 2048], f32, tag="trash")
                    nc.scalar.activation(
                        t[:, 0:g["w"]],
                        psum[:, 0:g["w"]],
                        mybir.ActivationFunctionType.Sigmoid,
                        bias=bias_sb[:, 0:1],
                        scale=float(-VER_ALPHA),
                        accum_out=outs[:, gi:gi + 1],
                    )
                else:
                    nc.vector.tensor_reduce(
                        outs[:, gi:gi + 1],
                        psum[:, 0:g["w"]],
                        axis=mybir.AxisListType.X,
                        op=mybir.AluOpType.min,
                    )
            nc.gpsimd.dma_start(out=out_d[:], in_=outs[:])
    nc.compile()
    return nc


def _augmented_f8(data):
    """fp8(e4m3) augmented operands for the verification GEMM (K=KV=70).

    u_i = (-2x_i, sqh_i, sql_i, sqll_i, 1, 1, 1)
    v_j = (x_j,   1,     1,     1,      sqh_j, sql_j, sqll_j)
    u.v = -2<x_i,x_j> + sq_i + sq_j = d2_ij, with |err| typically < ~5
    (dot-product quantization sigma ~1, sq-split err < 0.15).
    """
    import ml_dtypes

    f8 = ml_dtypes.float8_e4m3fn
    sq = np.einsum("ij,ij->i", data, data, dtype=np.float32)
    sh = sq.astype(f8)
    sl = (sq - sh.astype(np.float32)).astype(f8)
    sll = (sq - sh.astype(np.float32) - sl.astype(np.float32)).astype(f8)
    ones = np.ones((N, 1), f8)
    col = lambda a: a[:, None]
    U8 = np.concatenate(
        [(-2.0 * data).astype(f8), col(sh), col(sl), col(sll), ones, ones, ones],
        axis=1,
    )
    V8 = np.concatenate(
        [data.astype(f8), ones, ones, ones, col(sh), col(sl), col(sll)], axis=1
    )
    return U8, V8


def _ver_in_maps(U8, V8):
    """Per-core uv buffers for the verification launch."""
    VT = np.ascontiguousarray(V8.T)  # [KV, N]
    in_maps = []
    for c in range(NCORES):
        buf = np.empty((KV, VER_COLS), VT.dtype)
        for s in range(8):
            r = 8 * s + c
            buf[:, 128 * s:128 * (s + 1)] = U8[128 * r:128 * (r + 1)].T
            b0 = 128 * (r + 1)
            w = min(VER_W, N - b0)
            a = VER_B0 + VER_W * s
            if w > 0:
                buf[:, a:a + w] = VT[:, b0:b0 + w]
            if w < VER_W:  # pad with strictly sub-diagonal (safe) columns
                buf[:, a + w:a + VER_W] = VT[:, 0:VER_W - w]
            buf[:, VER_D0 + 128 * s:VER_D0 + 128 * (s + 1)] = VT[
                :, 128 * r:128 * (r + 1)
            ]
        buf[:, VER_VS0:VER_COLS] = VT[:, 1024:N]
        in_maps.append({"uv": buf})
    return in_maps


def _ver_decision(results, rho_t, delta_t):
    """True iff the device output certifies that every point is a center."""
    if not (rho_t < 1.0 - 1e-6):
        return False
    t2 = max(float(delta_t), 0.0) ** 2
    if t2 > VER_T2MAX:  # count certification only covers d2 below ~theta-err
        return False
    groups = _ver_groups()
    for c in range(NCORES):
        out = results[c]["vout"]  # [P, ng]
        if not np.all(np.isfinite(out)):
            return False
        for gi, g in enumerate(groups):
            if g["eng"] == "act":
                cnt = float(out[:, gi].astype(np.float64).sum())
                expect = 1024.0 if g["diag"] else 0.0
                if abs(cnt - expect) > 0.4:
                    return False
            else:
                if float(out[:, gi].min()) <= t2 + VER_MARGIN:
                    return False
    return True


def _f32r(ap):
    import concourse.mybir as mybir

    return ap.bitcast(mybir.dt.float32r)



def _pe_warmup(nc, tc, inp, psum_p, mybir, n_mm=8):
    """Dense garbage-matmul burst at launch start: runs while the input DMA
    streams, trips the PE HAM un-throttle (~3.4us sustained busy) so real
    matmuls run at 2.4 GHz instead of 1.2 GHz."""
    f32 = mybir.dt.float32
    warm = inp.tile([K, P + MM_N], mybir.dt.bfloat16)
    nc.gpsimd.memset(warm[:], 1.0)
    wps = psum_p.tile([P, FD], f32, tag="psum")
    for j in range(n_mm):
        nc.tensor.matmul(
            wps[:, (j % MM_PER_G) * MM_N:((j % MM_PER_G) + 1) * MM_N],
            warm[:, :P],
            warm[:, P:P + MM_N],
            start=True,
            stop=True,
        )


def _build_l12():
    """Merged count + rho launch: dc^2 is computed ON DEVICE.

    Every core counts the SAME sample (rows 0..1023 via the shared `uvc`
    lhsT, diag-free col groups), so each core independently derives an
    identical dc^2 - no collectives. The CDF interpolation runs as tiny
    [1,8] vector ops; a PE ones-matmul does the partition reduction and a
    K=1 fp32 matmul broadcasts -1/dc^2 to all partitions for the rho phase.
    `dvec` carries host-computed control-variate corrections (in counts)
    that cancel the row/col sampling bias of the fixed sample.
    """
    import concourse.mybir as mybir
    import concourse.tile as tile
    from concourse import bacc

    f32 = mybir.dt.float32
    nc = bacc.Bacc("TRN2", debug=False, enable_asserts=False)
    bf16 = mybir.dt.bfloat16
    uv_d = nc.dram_tensor("uv", [K, ROWS + N], bf16, kind="ExternalInput")
    uvc_d = nc.dram_tensor("uvc", [K, ROWS], bf16, kind="ExternalInput")
    thr_d = nc.dram_tensor("thr", [P, NT], f32, kind="ExternalInput")
    tvec_d = nc.dram_tensor("tvec", [1, NT], f32, kind="ExternalInput")
    dvec_d = nc.dram_tensor("dvec", [1, NT], f32, kind="ExternalInput")
    cnt_d = nc.dram_tensor("counts", [P, NT], f32, kind="ExternalOutput")
    rho_d = nc.dram_tensor("rho", [P, RB], f32, kind="ExternalOutput")

    with tile.TileContext(nc) as tc:
        with (
            tc.tile_pool(name="inp", bufs=1) as inp,
            tc.tile_pool(name="stat", bufs=1) as stat,
            tc.tile_pool(name="trash", bufs=2) as trash_p,
            tc.tile_pool(name="psum", bufs=2, space="PSUM") as psum_p,
        ):
            uv_sb = inp.tile([K, ROWS + N], bf16)
            uvc_sb = inp.tile([K, ROWS], bf16)
            nc.sync.dma_start(out=uvc_sb[:], in_=uvc_d[:])
            for _g in (1, 2, 3):  # count-phase cols first; group 0 only for rho
                _a = ROWS + _g * FD
                nc.sync.dma_start(
                    out=uv_sb[:, _a:_a + FD], in_=uv_d[:, _a:_a + FD]
                )
            nc.sync.dma_start(out=uv_sb[:, 0:ROWS], in_=uv_d[:, 0:ROWS])
            nc.sync.dma_start(
                out=uv_sb[:, ROWS:ROWS + FD], in_=uv_d[:, ROWS:ROWS + FD]
            )
            thr_sb = inp.tile([P, NT], f32)
            nc.gpsimd.dma_start(out=thr_sb[:], in_=thr_d[:])
            tdv_sb = inp.tile([1, 2 * NT], f32)
            nc.gpsimd.dma_start(out=tdv_sb[:, 0:NT], in_=tvec_d[:])
            nc.gpsimd.dma_start(out=tdv_sb[:, NT:2 * NT], in_=dvec_d[:])
            cnts = stat.tile([P, NT], f32)
            warmact = stat.tile([P, 1], f32)
            nc.scalar.activation(
                warmact[:], thr_sb[:, 0:1],
                mybir.ActivationFunctionType.Sigmoid, bias=0.0, scale=1.0,
            )

            # ---- phase 1: counts over the shared sample -----------------
            for b, (m, g) in enumerate(L1_GROUPS):
                psum = psum_p.tile([P, FD], f32, tag="psum")
                for j in range(L1_W // MM_N):
                    nc.tensor.matmul(
                        psum[:, j * MM_N:(j + 1) * MM_N],
                        uvc_sb[:, m * P:(m + 1) * P],
                        uv_sb[:, ROWS + g * FD + j * MM_N: ROWS + g * FD + (j + 1) * MM_N],
                        start=True,
                        stop=True,
                    )
                t = trash_p.tile([P, L1_W], f32, tag="cntrash")
                nc.scalar.activation(
                    t[:],
                    psum[:, 0:L1_W],
                    mybir.ActivationFunctionType.Sigmoid,
                    bias=thr_sb[:, b:b + 1],
                    scale=float(-SIG_ALPHA),
                    accum_out=cnts[:, b:b + 1],
                )
            nc.gpsimd.dma_start(out=cnt_d[:], in_=cnts[:])

            # ---- phase 2: dc^2 from counts (identical on every core) ----
            ones_col = stat.tile([P, 1], f32)
            nc.vector.memset(ones_col[:], 1.0)
            ps_tot = psum_p.tile([1, NT], f32, tag="psum")
            nc.tensor.matmul(ps_tot[:], ones_col[:], cnts[:], start=True, stop=True)
            w = stat.tile([1, 8 * NT], f32)  # scratch lanes along free dim
            q = w[:, 0:NT]
            nc.vector.tensor_tensor(
                out=q, in0=ps_tot[:], in1=tdv_sb[:, NT:2 * NT],
                op=mybir.AluOpType.subtract,
            )
            NB_ = NT - 1
            a_ = w[:, NT:NT + NB_]
            nc.vector.tensor_scalar(
                out=a_, in0=q[:, 0:NB_], scalar1=CSTAR, scalar2=None,
                op0=mybir.AluOpType.is_le,
            )
            b_ = w[:, 2 * NT:2 * NT + NB_]
            nc.vector.tensor_scalar(
                out=b_, in0=q[:, 1:NT], scalar1=CSTAR, scalar2=None,
                op0=mybir.AluOpType.is_gt,
            )
            sel = w[:, 3 * NT:3 * NT + NB_]
            nc.vector.tensor_tensor(out=sel, in0=a_, in1=b_, op=mybir.AluOpType.mult)
            den = w[:, 4 * NT:4 * NT + NB_]
            nc.vector.tensor_tensor(
                out=den, in0=q[:, 1:NT], in1=q[:, 0:NB_],
                op=mybir.AluOpType.subtract,
            )
            rec = w[:, 5 * NT:5 * NT + NB_]
            nc.vector.reciprocal(rec, den)
            num = w[:, 6 * NT:6 * NT + NB_]
            nc.vector.tensor_scalar(
                out=num, in0=q[:, 0:NB_], scalar1=-1.0, scalar2=CSTAR,
                op0=mybir.AluOpType.mult, op1=mybir.AluOpType.add,
            )
            fr = w[:, 7 * NT:7 * NT + NB_]
            nc.vector.tensor_tensor(out=fr, in0=num, in1=rec, op=mybir.AluOpType.mult)
            nc.vector.tensor_scalar(
                out=fr, in0=fr, scalar1=float(DC2_STEP), scalar2=None,
                op0=mybir.AluOpType.mult,
            )
            nc.vector.tensor_tensor(
                out=fr, in0=fr, in1=tdv_sb[:, 0:NB_], op=mybir.AluOpType.add
            )
            nc.vector.tensor_tensor(out=fr, in0=fr, in1=sel, op=mybir.AluOpType.mult)
            sc = stat.tile([1, 4], f32)
            nc.vector.tensor_reduce(
                sc[:, 0:1], fr[:], axis=mybir.AxisListType.X, op=mybir.AluOpType.add
            )
            nc.vector.tensor_reduce(
                sc[:, 1:2], sel[:], axis=mybir.AxisListType.X, op=mybir.AluOpType.add
            )
            # guard: if no bracket, fall back to the grid center
            nc.vector.tensor_scalar(
                out=sc[:, 2:3], in0=sc[:, 1:2], scalar1=float(-DC2_CENTER),
                scalar2=float(DC2_CENTER), op0=mybir.AluOpType.mult,
                op1=mybir.AluOpType.add,
            )
            nc.vector.tensor_tensor(
                out=sc[:, 0:1], in0=sc[:, 0:1], in1=sc[:, 2:3],
                op=mybir.AluOpType.add,
            )
            nc.vector.reciprocal(sc[:, 3:4], sc[:, 0:1])
            nc.vector.tensor_scalar(
                out=sc[:, 3:4], in0=sc[:, 3:4], scalar1=-1.0, scalar2=None,
                op0=mybir.AluOpType.mult,
            )
            ones_row = stat.tile([1, P], f32)
            nc.vector.memset(ones_row[:], 1.0)
            ps_b = psum_p.tile([P, 1], f32, tag="psum")
            nc.tensor.matmul(ps_b[:], ones_row[:], sc[:, 3:4], start=True, stop=True)
            scl_sb = stat.tile([P, 1], f32)
            nc.vector.tensor_copy(scl_sb[:], ps_b[:])

            # ---- phase 3: rho ------------------------------------------
            parts = stat.tile([P, RB * G], f32)
            rho_sb = stat.tile([P, RB], f32)
            for m in range(RB):
                for g in range(G):
                    psum = psum_p.tile([P, FD], f32, tag="psum")
                    for j in range(MM_PER_G):
                        nc.tensor.matmul(
                            psum[:, j * MM_N:(j + 1) * MM_N],
                            uv_sb[:, m * P:(m + 1) * P],
                            uv_sb[:, ROWS + g * FD + j * MM_N: ROWS + g * FD + (j + 1) * MM_N],
                            start=True,
                            stop=True,
                        )
                    t = trash_p.tile([P, FD], f32, tag="trash")
                    q2 = m * G + g
                    nc.scalar.activation(
                        t[:],
                        psum[:],
                        mybir.ActivationFunctionType.Exp,
                        bias=0.0,
                        scale=scl_sb[:, 0:1],
                        accum_out=parts[:, q2:q2 + 1],
                    )
                nc.vector.tensor_reduce(
                    rho_sb[:, m:m + 1],
                    parts[:, m * G:(m + 1) * G],
                    axis=mybir.AxisListType.X,
                    op=mybir.AluOpType.add,
                )
            nc.sync.dma_start(out=rho_d[:], in_=rho_sb[:])
    nc.compile()
    return nc


def _build_l3():
    """Delta pass on rho-sorted data (round-robin block interleaving).

    Core c holds sorted row-blocks b = 8m + c (m = 0..7). For local block m:
      boundary col-group g_b = m//2, window base w_lo = 1024*(m%2)
      (cutoffs of every core's block-m rows lie in [w_lo, w_lo+1024) of
      group g_b, ties aside - those are patched on host).
    Structure per block:
      groups g < g_b:                plain min-reduce of the whole group
      boundary prefix [0, w_lo):     plain min-reduce (odd m only)
      boundary window [w_lo,+1024):  penalty mask (iota >= cutrel)*BIG, add,
                                     min-reduce
      columns beyond w_lo+1024 and groups g > g_b: skipped entirely.
    """
    import concourse.mybir as mybir
    import concourse.tile as tile
    from concourse import bacc

    f32 = mybir.dt.float32
    nc = bacc.Bacc("TRN2", debug=False, enable_asserts=False)
    bf16 = mybir.dt.bfloat16
    uv_d = nc.dram_tensor("uv", [K, ROWS + N], bf16, kind="ExternalInput")
    cut_d = nc.dram_tensor("cut", [P, RB], f32, kind="ExternalInput")
    iota_d = nc.dram_tensor("iota", [P, WW], f32, kind="ExternalInput")
    dmin_d = nc.dram_tensor("dmin", [P, RB * NCOL], f32, kind="ExternalOutput")

    with tile.TileContext(nc) as tc:
        with (
            tc.tile_pool(name="inp", bufs=1) as inp,
            tc.tile_pool(name="stat", bufs=1) as stat,
            tc.tile_pool(name="trash", bufs=3) as trash_p,
            tc.tile_pool(name="pen", bufs=3) as pen_p,
            tc.tile_pool(name="psum", bufs=2, space="PSUM") as psum_p,
        ):
            uv_sb = inp.tile([K, ROWS + N], bf16)
            nc.sync.dma_start(out=uv_sb[:, 0:ROWS], in_=uv_d[:, 0:ROWS])
            for _g in range(G):
                _a = ROWS + _g * FD
                nc.sync.dma_start(
                    out=uv_sb[:, _a:_a + FD], in_=uv_d[:, _a:_a + FD]
                )
            cut_sb = inp.tile([P, RB], f32)
            nc.gpsimd.dma_start(out=cut_sb[:], in_=cut_d[:])
            iota_sb = inp.tile([P, WW], f32)
            nc.gpsimd.dma_start(out=iota_sb[:], in_=iota_d[:])
            dmin_sb = stat.tile([P, RB * NCOL], f32)

            for m in range(RB):
                gb = m // 2
                w_lo = WW * (m % 2)
                pen = pen_p.tile([P, WW], f32, tag="pen")
                # cutrel (host-clamped to [0, WW]) is relative to w_lo
                nc.vector.tensor_scalar(
                    out=pen[:],
                    in0=iota_sb[:],
                    scalar1=cut_sb[:, m:m + 1],
                    scalar2=PEN_BIG,
                    op0=mybir.AluOpType.is_ge,
                    op1=mybir.AluOpType.mult,
                )
                for g in range(gb + 1):
                    ncols = FD if g < gb else w_lo + WW
                    psum = psum_p.tile([P, FD], f32, tag="psum")
                    for j in range(ncols // MM_N):
                        nc.tensor.matmul(
                            psum[:, j * MM_N:(j + 1) * MM_N],
                            uv_sb[:, m * P:(m + 1) * P],
                            uv_sb[:, ROWS + g * FD + j * MM_N: ROWS + g * FD + (j + 1) * MM_N],
                            start=True,
                            stop=True,
                        )
                    q = m * NCOL + g
                    if g < gb:
                        nc.vector.tensor_reduce(
                            dmin_sb[:, q:q + 1],
                            psum[:],
                            axis=mybir.AxisListType.X,
                            op=mybir.AluOpType.min,
                        )
                    else:
                        if w_lo > 0:
                            nc.vector.tensor_reduce(
                                dmin_sb[:, q:q + 1],
                                psum[:, 0:w_lo],
                                axis=mybir.AxisListType.X,
                                op=mybir.AluOpType.min,
                            )
                        t = trash_p.tile([P, WW], f32, tag="trash")
                        nc.vector.tensor_tensor(
                            out=t[:],
                            in0=psum[:, w_lo:w_lo + WW],
                            in1=pen[:],
                            op=mybir.AluOpType.add,
                        )
                        nc.vector.tensor_reduce(
                            dmin_sb[:, m * NCOL + G:m * NCOL + G + 1],
                            t[:],
                            axis=mybir.AxisListType.X,
                            op=mybir.AluOpType.min,
                        )
            nc.gpsimd.dma_start(out=dmin_d[:], in_=dmin_sb[:])
    nc.compile()
    return nc


_BUILDERS = {"l12": _build_l12, "l3": _build_l3, "ver": _build_ver}


def _get_program(name):
    if name not in _programs:
        _programs[name] = _BUILDERS[name]()
    return _programs[name]


TIMINGS = []  # (name, exec_time_ns) per launch, appended by _run


def _run(name, in_maps, trace=None):
    from concourse.bass_utils import run_bass_kernel_spmd

    if trace is None:
        trace = bool(int(os.environ.get("KERNEL_TRACE", "0")))
    nc = _get_program(name)
    res = run_bass_kernel_spmd(
        nc, in_maps, core_ids=list(range(NCORES)), trace=trace
    )
    TIMINGS.append((name, res.exec_time_ns))
    return res


def _augmented(data):
    """U (lhs rows) and V (rhs cols) of the K=68 augmented distance GEMM.

    bf16 operands with sq split into a bf16 hi+lo pair: d2 error ~0.04 abs
    (~5e-4 relative at the dc^2 scale), far inside every decision margin.
    """
    import ml_dtypes

    bf = ml_dtypes.bfloat16
    sq = np.einsum("ij,ij->i", data, data, dtype=np.float32).astype(np.float32)
    sqh = sq.astype(bf)
    sql = (sq - sqh.astype(np.float32)).astype(bf)
    ones = np.ones((N, 1), bf)
    zcol = lambda a: a[:, None]
    U = np.concatenate(
        [(-2.0 * data).astype(bf), zcol(sqh), zcol(sql), ones, ones], axis=1
    )
    V = np.concatenate(
        [data.astype(bf), ones, ones, zcol(sqh), zcol(sql)], axis=1
    )
    return U, V, sq


def _erf(x):
    """Abramowitz-Stegun 7.1.26 vectorized erf (|err| < 1.5e-7)."""
    s = np.sign(x)
    x = np.abs(x)
    t = 1.0 / (1.0 + 0.3275911 * x)
    y = 1.0 - (
        ((((1.061405429 * t - 1.453152027) * t) + 1.421413741) * t - 0.284496736)
        * t
        + 0.254829592
    ) * t * np.exp(-x * x)
    return s * y


def _phi(z):
    return 0.5 * (1.0 + _erf(z / np.sqrt(2.0)))


NGRID = 256


def _cv_corrections(sq):
    """Control-variate count corrections for the fixed count sample.

    Model P(d2 < t | sq_i, sq_j) ~ Phi((t - sq_i - sq_j)/(2 sqrt(sq_i sq_j/D)))
    and subtract the predicted row/col selection bias of the sampled
    rows/cols relative to the full point set.
    """
    sq64 = sq.astype(np.float64)
    step = N // NGRID
    grid = np.sort(sq64)[step // 2::step][:NGRID]

    def h(t, svals):
        s = svals[:, None]
        sp = grid[None, :]
        z = (t - s - sp) / (2.0 * np.sqrt(np.maximum(s * sp, 1e-9) / D))
        return _phi(z).mean(axis=1)

    dvec = np.zeros(NT)
    for b, (m, g) in enumerate(L1_GROUPS):
        t = float(DC2_GRID[b])
        h_all = h(t, grid).mean()
        d_row = h(t, sq64[m * P:(m + 1) * P]).mean() - h_all
        d_col = h(t, sq64[g * FD:g * FD + L1_W]).mean() - h_all
        dvec[b] = (d_row + d_col) * (P * L1_W)
    return dvec.astype(np.float32).reshape(1, NT)


def _interp_dc2(counts_by_core):
    """counts_by_core: list of [P, NT] arrays -> dc^2 via CDF interpolation."""
    M = float(N) * float(N)
    k_pos = PCT / 100.0 * (M - 1.0)
    p_off = (k_pos - N) / (M - N)  # diag cells (d2=0) all fall below any t_b

    tot = np.zeros(NT, np.float64)
    denom = np.zeros(NT, np.float64)
    for c in range(NCORES):
        cc = counts_by_core[c].astype(np.float64).sum(axis=0)  # [NT]
        for b, (m, g) in enumerate(L1_GROUPS):
            row0 = c * ROWS + m * P
            off = row0 - g * FD
            has_diag = 0 <= off <= L1_W - P
            tot[b] += cc[b] - (P if has_diag else 0)
            denom[b] += P * L1_W - (P if has_diag else 0)
    p_hat = tot / denom
    # p_hat should be increasing in b; enforce monotonicity for safety
    p_mono = np.maximum.accumulate(p_hat)
    if not (p_mono[0] <= p_off <= p_mono[-1]):
        return None  # bracket miss -> caller falls back to exact host path
    b_hi = int(np.searchsorted(p_mono, p_off, side="left"))
    if b_hi == 0:
        return float(DC2_GRID[0])
    b_lo = b_hi - 1
    p_lo, p_hi_v = p_mono[b_lo], p_mono[b_hi]
    frac = 0.0 if p_hi_v <= p_lo else (p_off - p_lo) / (p_hi_v - p_lo)
    return float(DC2_GRID[b_lo] + frac * (DC2_GRID[b_hi] - DC2_GRID[b_lo]))


def _host_fallback(data, rho_t, delta_t):
    """Pure-numpy reference path (only used if device assumptions break)."""
    data = np.asarray(data, np.float32)
    sq = np.sum(data * data, axis=1)
    d2 = sq[:, None] + sq[None, :] - 2.0 * (data @ data.T)
    dist = np.sqrt(np.maximum(d2, 0.0), dtype=np.float32)
    dc = np.percentile(dist, PCT)
    rho = np.exp(-((dist / dc) ** 2)).sum(axis=1).astype(np.float32)
    higher = rho[None, :] > rho[:, None]
    masked = np.where(higher, dist, np.inf)
    delta_m = masked.min(axis=1)
    nhd_m = masked.argmin(axis=1)
    has = higher.any(axis=1)
    delta = np.where(has, delta_m, dist.max(axis=1))
    nhd = np.where(has, nhd_m, np.arange(N))
    return _finish_labels(rho, delta, nhd, rho_t, delta_t)


def _finish_labels(rho, delta, nhd, rho_t, delta_t):
    is_center = (rho > rho_t) & (delta > delta_t)
    center_rank = np.cumsum(is_center.astype(np.int32)) - 1
    labels = np.where(is_center, center_rank, -1).astype(np.int32)
    order = np.argsort(-rho, kind="stable")
    for i in order:
        if labels[i] < 0:
            labels[i] = labels[nhd[i]]
    return labels


def kernel(data, rho_threshold, delta_threshold):
    data = np.ascontiguousarray(np.asarray(data, dtype=np.float32))
    assert data.shape == (N, D)
    rho_t = float(np.asarray(rho_threshold))
    delta_t = float(np.asarray(delta_threshold))

    # ---- fast path: certify "every point is a center" ------------------
    U8, V8 = _augmented_f8(data)
    rver = _run("ver", _ver_in_maps(U8, V8))
    if _ver_decision(rver.results, rho_t, delta_t):
        return np.arange(N, dtype=np.int32)

    U, V, sq = _augmented(data)
    VT = V.T  # [K, N]

    # ---- L12: counts -> on-device dc^2 -> rho (single launch) ----------
    thr = np.broadcast_to(
        (SIG_ALPHA * DC2_GRID).astype(np.float32)[None, :], (P, NT)
    ).copy()
    tvec = DC2_GRID.astype(np.float32).reshape(1, NT)
    dvec = _cv_corrections(sq)
    uvc = np.ascontiguousarray(np.concatenate([U[0:ROWS].T, VT], axis=1)[:, 0:ROWS])
    in_maps = [
        {
            "uv": np.ascontiguousarray(
                np.concatenate([U[c * ROWS:(c + 1) * ROWS].T, VT], axis=1)
            ),
            "uvc": uvc,
            "thr": thr,
            "tvec": tvec,
            "dvec": dvec,
        }
        for c in range(NCORES)
    ]
    r12 = _run("l12", in_maps)

    # validate the on-device dc interpolation from the counts output
    q = r12.results[0]["counts"].astype(np.float64).sum(axis=0) - dvec[0].astype(
        np.float64
    )
    brackets = [
        b for b in range(NT - 1) if q[b] <= CSTAR < q[b + 1]
    ]
    if len(brackets) != 1 or not np.all(np.diff(q) > 0):
        return _host_fallback(data, rho_t, delta_t)

    rho = np.empty(N, np.float32)
    for c in range(NCORES):
        out = r12.results[c]["rho"]  # [P, RB]
        rho[c * ROWS:(c + 1) * ROWS] = out.T.reshape(-1)
    if not np.all(np.isfinite(rho)) or rho.min() < 0.5 or rho.max() > N + 1:
        return _host_fallback(data, rho_t, delta_t)

    # ---- host: sort by rho desc; prefix cutoffs ------------------------
    order = np.argsort(-rho, kind="stable")
    rho_sorted = rho[order]
    # c_i = #points with rho strictly greater (ties excluded)
    cuts = np.searchsorted(-rho_sorted, -rho_sorted, side="left").astype(np.int64)

    data_p = data[order]
    sq_p = sq[order]
    Up = U[order]
    Vp = V[order]
    rhs_p = np.ascontiguousarray(Vp.T)

    # round-robin block interleave: core c <- sorted blocks 8m + c
    NB = N // P  # 64 sorted row-blocks
    blk_rows = np.arange(N).reshape(NB, P)
    core_rows = [blk_rows[np.arange(RB) * NCORES + c].reshape(-1) for c in range(NCORES)]

    iota_in = np.broadcast_to(
        np.arange(WW, dtype=np.float32)[None, :], (P, WW)
    ).copy()
    in_maps = []
    for c in range(NCORES):
        rows = core_rows[c]
        cutrel = np.empty((P, RB), np.float32)
        for m in range(RB):
            base = (m // 2) * FD + WW * (m % 2)
            cutrel[:, m] = np.clip(cuts[rows[m * P:(m + 1) * P]] - base, 0, WW)
        in_maps.append(
            {
                "uv": np.ascontiguousarray(
                    np.concatenate([Up[rows].T, rhs_p], axis=1)
                ),
                "cut": cutrel,
                "iota": iota_in,
            }
        )
    r3 = _run("l3", in_maps)
    # dmin[i] holds per-source minima; dcol[k] = (col_base, col_len) of source k
    dmin = np.full((N, NCOL), np.inf, np.float32)
    for c in range(NCORES):
        out = r3.results[c]["dmin"]  # [P, RB*NCOL]
        rows = core_rows[c]
        for m in range(RB):
            gb = m // 2
            w_lo = WW * (m % 2)
            blk = rows[m * P:(m + 1) * P]
            for g in range(gb):
                dmin[blk, g] = out[:, m * NCOL + g]
            if w_lo > 0:
                dmin[blk, gb] = out[:, m * NCOL + gb]
            dmin[blk, G] = out[:, m * NCOL + G]

    # ---- host: delta, fallback rows, centers, nhd (lazy), labels -------
    delta2_sorted = dmin.min(axis=1)

    # rho-tie rows whose cutoff dips below their block's boundary group: the
    # device's full-group reduce included a few extra columns; fix exactly.
    win_base = ((np.arange(N) // P) // NCORES) * WW  # 1024*m per sorted row
    straddle_fix = {}
    for i in np.nonzero(cuts < win_base)[0]:
        cut = int(cuts[i])
        if cut == 0:
            delta2_sorted[i] = np.inf
            continue
        d2row = sq_p[i] + sq_p[:cut] - 2.0 * (data_p[:cut] @ data_p[i])
        j = int(np.argmin(d2row))
        delta2_sorted[i] = d2row[j]
        straddle_fix[i] = j

    empty = delta2_sorted >= EMPTY_SENTINEL  # no higher-density point
    delta_sorted = np.sqrt(np.maximum(delta2_sorted, 0.0), dtype=np.float32)
    for i in np.nonzero(empty)[0]:
        d2row = sq_p[i] + sq_p - 2.0 * (data_p @ data_p[i])
        delta_sorted[i] = np.sqrt(max(float(np.max(np.maximum(d2row, 0.0))), 0.0))

    delta = np.empty(N, np.float32)
    delta[order] = delta_sorted

    is_center = (rho > rho_t) & (delta > delta_t)
    center_rank = np.cumsum(is_center.astype(np.int32)) - 1
    labels = np.where(is_center, center_rank, -1).astype(np.int32)

    need_nhd = ~is_center[order]  # sorted positions whose label must propagate
    nhd = np.arange(N, dtype=np.int64)  # default: self (matches reference)
    for i in np.nonzero(need_nhd)[0]:
        if empty[i]:
            continue  # nhd stays self, as in reference
        if i in straddle_fix:
            nhd[order[i]] = order[straddle_fix[i]]
            continue
        k = int(np.argmin(dmin[i]))
        m = (i // P) // NCORES
        gb = m // 2
        w_lo = WW * (m % 2)
        if k == G:
            c0, clen = gb * FD + w_lo, WW
        elif k == gb:
            c0, clen = gb * FD, w_lo
        else:
            c0, clen = k * FD, FD
        end_local = int(np.clip(cuts[i] - c0, 0, clen))
        cols = slice(c0, c0 + end_local)
        d2part = sq_p[i] + sq_p[cols] - 2.0 * (data_p[cols] @ data_p[i])
        j_local = int(np.argmin(d2part))
        nhd[order[i]] = order[c0 + j_local]

    for i in order:
        if labels[i] < 0:
            labels[i] = labels[nhd[i]]
    return labels.astype(np.int32)



# revision 15
# speedup vs baseline: 4.1275x; 4.1232x over previous
"""CFSDP (density-peaks clustering) on 8 Trainium2 NeuronCores.

Pipeline (N=8192 points, D=64, row-sharded 1024 rows/core):
  d2(i,j) = ||xi-xj||^2 via one K=66 augmented matmul per tile:
      u_i = (-2*x_i, sq_i, 1),  v_j = (x_j, 1, sq_j),  d2 = u_i . v_j
  All O(N^2) math runs on squared distances (sqrt is monotone, so order
  stats / argmin / percentile commute with it):
    L1: count(d2 < t_b) for 16 thresholds around the predicted 2%-quantile
        (ACT sigmoid step fn + accumulate) -> host interpolates dc^2.
    L2: rho_i = sum_j exp(-d2_ij/dc^2) (ACT Exp + accumulate, scale from SBUF).
    host: stable-sort rows by rho desc; "higher density" mask becomes a
        per-row prefix of the sorted column order.
    L3: delta_i^2 = min over prefix window of d2 (vector.tensor_mask_reduce,
        per-partition index window, on negated-d2 PSUM tiles).
  Host finishes: delta fallback (row max) for top-density rows, nhd argmin
  (lazy, only for non-center points), center ranks, label propagation scan.
"""

import os
import numpy as np

N = 8192
D = 64
NCORES = 8
ROWS = N // NCORES          # 1024 rows per core
P = 128                     # partitions
RB = ROWS // P              # 8 row-blocks per core
FD = 2048                   # free-dim group (4 PSUM banks)
G = N // FD                 # 4 col-groups per row
K = D + 4                   # 68 (augmented contraction dim, sq split hi+lo)
MM_N = 512                  # cols per matmul (one PSUM bank output)
MM_PER_G = FD // MM_N       # 4

NT = 4                      # percentile-count thresholds
L1_W = 1024                 # cols counted per threshold
DC2_CENTER = 86.2           # chi^2_64-predicted 2%-quantile of d2 (randn data)
DC2_GRID = (DC2_CENTER * (1.0 + (np.arange(NT) - (NT - 1) / 2) * 0.023)).astype(
    np.float64
)                           # +-3.5% bracket, 2.3% spacing
SIG_ALPHA = 2.0e4           # sigmoid step sharpness (soft window ~1e-3 in d2)
PCT = 2.0
FLT_MAX = float(np.finfo(np.float32).max)
PEN_BIG = 1e38              # penalty added beyond the prefix cutoff
PEN_ALPHA = 1e31            # relu penalty slope (ACT-built mask, L3)
WW = 1024                   # L3 boundary mask window width
NCOL = G + 1                # L3 output cols per block (G group slots + window)
EMPTY_SENTINEL = 1e37       # accum >= this => empty prefix window

# threshold b is counted on group (m, g) of every core (1/16 of the matrix
# per threshold => ~4.2M samples each; different rows+cols per threshold)
L1_GROUPS = [(b % RB, 1 + b % (G - 1)) for b in range(NT)]  # g>0: diag-free
DC2_STEP = float(DC2_CENTER * 0.023)
M_TOT = float(N) * float(N)
K_POS = PCT / 100.0 * (M_TOT - 1.0)
P_OFF = (K_POS - N) / (M_TOT - N)      # diag-free target CDF
CSTAR = float(P_OFF * P * L1_W)        # target count over the device sample

_programs: dict = {}

# ---------------------------------------------------------------------------
# Fast path: all-centers verification.
#
# For this problem's regime (randn data, thresholds 0.5) every point is a
# cluster center: rho_i >= 1 (diagonal exp(0) term) > rho_t, and
# delta_i >= min_{j!=i} dist_ij ~ 6.1 >> delta_t.  When that holds, labels
# are exactly arange(N) regardless of rho ordering.  One launch certifies it:
# compute the (upper-triangle) pairwise d2 and check no off-diagonal pair is
# within the threshold.  Groups of PSUM output are drained alternately by the
# ACT engine (sigmoid step counts, which also absorb the diagonal cells: each
# contributes ~1, totalling exactly 1024/core) and the DVE (min-reduce over
# diag-free groups).  Host passes iff rho_t < 1, every DVE min is above
# delta_t^2 + margin, and every ACT count matches its expected value.
# Otherwise we fall back to the full pipeline below.
#
# Per-core layout (core c, slot s = row-block r = 8s + c, 128 rows each):
#   U  [0:1024)      lhsT for the 8 slots
#   B  [1024:8192)   per-slot boundary cols  [128(r+1), +896) of V (padded
#                    with safely sub-diagonal cols where it runs off the end)
#   D  [8192:9216)   per-slot diagonal cols  [128r, +128) of V
#   VS [9216:16384)  shared V cols [1024:8192); slot s's tail reads
#                    VS[1024s : 7168)  (global cols [1024(s+1), 8192))
# Coverage of block r: [128r, 8192) with no gaps (tail start 1024(s+1)
# always <= boundary end 128(r+1)+896), so every upper pair is seen >= once;
# duplicates and sub-diagonal pad cells are true off-diagonal pairs and
# cannot fake a pass.
# ---------------------------------------------------------------------------

KV = 70                     # fp8 contraction: x(64) + sq hi/lo/lolo(3) + ones(3)
VER_ALPHA = 2.0             # sigmoid step sharpness (in d2 units)
VER_THETA = 20.0            # count threshold: flags any pair with d2 < ~20
VER_W = 896                 # boundary width: covers slot spread 7*128
VER_U0, VER_B0, VER_D0, VER_VS0 = 0, 1024, 8192, 9216
VER_COLS = 16384
VER_MARGIN = 14.0           # required gap above delta_t^2 (fp8 d2 err budget)
VER_T2MAX = 6.0             # max delta_t^2 the count certification covers


def _ver_groups():
    """Static PSUM-group schedule shared by the builder and the host check.

    Returns a list of dicts: tiles = [(slot, uv_col, width)], w = total
    width, eng = 'act' (sigmoid count) or 'dve' (min-reduce), diag flag.
    """
    groups = []
    for pair in range(4):
        tiles = []
        for s in (2 * pair, 2 * pair + 1):
            off = VER_B0 + VER_W * s
            tiles.append((s, off, 512))
            tiles.append((s, off + 512, 384))
        groups.append(dict(kind="b", tiles=tiles, w=1792, diag=False))
    groups.append(
        dict(
            kind="d",
            tiles=[(s, VER_D0 + 128 * s, 128) for s in range(8)],
            w=1024,
            diag=True,
        )
    )
    tail = []
    for s in range(6, -1, -1):  # descending s matches the VS DMA chunk order
        for k in range(14 - 2 * s):
            tail.append((s, VER_VS0 + 1024 * s + 512 * k, 512))
    for i in range(0, len(tail), 4):
        groups.append(dict(kind="t", tiles=tail[i:i + 4], w=2048, diag=False))
    # engine assignment: diag group must be ACT; balance projected busy time
    # (measured: ACT (w+352)/1.2 ns, DVE reduce ~0.92 elem/ns from PSUM)
    a_ns = d_ns = 0.0
    for g in groups:
        act_cost = (g["w"] + 352) / 1.2
        dve_cost = g["w"] / 0.92
        if g["diag"] or (not g["diag"] and a_ns + act_cost <= d_ns + dve_cost):
            g["eng"] = "act"
            a_ns += act_cost
        else:
            g["eng"] = "dve"
            d_ns += dve_cost
    return groups


def _build_ver():
    import concourse.mybir as mybir
    import concourse.tile as tile
    from concourse import bacc

    f32 = mybir.dt.float32
    f8 = mybir.dt.float8e4
    groups = _ver_groups()
    ng = len(groups)
    nc = bacc.Bacc("TRN2", debug=False, enable_asserts=False)
    uv_d = nc.dram_tensor("uv", [KV, VER_COLS], f8, kind="ExternalInput")
    out_d = nc.dram_tensor("vout", [P, ng], f32, kind="ExternalOutput")

    with tile.TileContext(nc) as tc:
        with (
            tc.tile_pool(name="inp", bufs=1) as inp,
            tc.tile_pool(name="stat", bufs=1) as stat,
            tc.tile_pool(name="trash", bufs=2) as trash_p,
            tc.tile_pool(name="psum", bufs=2, space="PSUM") as psum_p,
        ):
            uv_sb = inp.tile([KV, VER_COLS], f8)
            _pe_warmup(nc, tc, inp, psum_p, mybir)
            # DMA in consumption order, alternating the two free queues
            # (each DMA_DIRECT2D issue costs ~0.6-1us of queue time; the
            # stream rate ~90GB/s barely exceeds PE consumption in fp8, so
            # chunks release dependencies just ahead of the matmuls)
            chunks = [
                (0, 1920),                 # U + boundary slot 0
                (1920, 896),               # boundary slot 1
                (2816, 1792),              # boundary pair 1
                (4608, 1792),              # boundary pair 2
                (6400, 1792),              # boundary pair 3
                (VER_D0, 1024),            # diag cols
                (VER_VS0 + 5120, 2048),    # VS chunks, descending
                (VER_VS0 + 3072, 2048),
                (VER_VS0 + 1024, 2048),
                (VER_VS0, 1024),
            ]
            for i, (a, w) in enumerate(chunks):
                eng = nc.sync if i % 2 == 0 else nc.gpsimd
                eng.dma_start(out=uv_sb[:, a:a + w], in_=uv_d[:, a:a + w])

            outs = stat.tile([P, ng], f32)
            bias_sb = stat.tile([P, 1], f32)
            nc.vector.memset(bias_sb[:], float(VER_ALPHA * VER_THETA))
            warmact = stat.tile([P, 2], f32)
            nc.vector.memset(warmact[:, 0:1], 0.0)
            nc.scalar.activation(
                warmact[:, 1:2], warmact[:, 0:1],
                mybir.ActivationFunctionType.Sigmoid, bias=bias_sb[:, 0:1],
                scale=1.0,
            )
            for gi, g in enumerate(groups):
                psum = psum_p.tile([P, 2048], f32, tag="psum")
                col = 0
                for (s, off, wid) in g["tiles"]:
                    nc.tensor.matmul(
                        psum[:, col:col + wid],
                        uv_sb[:, 128 * s:128 * (s + 1)],
                        uv_sb[:, off:off + wid],
                        start=True,
                        stop=True,
                    )
                    col += wid
                if g["eng"] == "act":
                    t = trash_p.tile([P, 2048], f32, tag="trash")
                    nc.scalar.activation(
                        t[:, 0:g["w"]],
                        psum[:, 0:g["w"]],
                        mybir.ActivationFunctionType.Sigmoid,
                        bias=bias_sb[:, 0:1],
                        scale=float(-VER_ALPHA),
                        accum_out=outs[:, gi:gi + 1],
                    )
                else:
                    nc.vector.tensor_reduce(
                        outs[:, gi:gi + 1],
                        psum[:, 0:g["w"]],
                        axis=mybir.AxisListType.X,
                        op=mybir.AluOpType.min,
                    )
            nc.gpsimd.dma_start(out=out_d[:], in_=outs[:])
    nc.compile()
    return nc


def _augmented_f8(data):
    """fp8(e4m3) augmented operands for the verification GEMM (K=KV=70).

    u_i = (-2x_i, sqh_i, sql_i, sqll_i, 1, 1, 1)
    v_j = (x_j,   1,     1,     1,      sqh_j, sql_j, sqll_j)
    u.v = -2<x_i,x_j> + sq_i + sq_j = d2_ij, with |err| typically < ~5
    (dot-product quantization sigma ~1, sq-split err < 0.15).
    """
    import ml_dtypes

    f8 = ml_dtypes.float8_e4m3fn
    sq = np.einsum("ij,ij->i", data, data, dtype=np.float32)
    sh = sq.astype(f8)
    sl = (sq - sh.astype(np.float32)).astype(f8)
    sll = (sq - sh.astype(np.float32) - sl.astype(np.float32)).astype(f8)
    ones = np.ones((N, 1), f8)
    col = lambda a: a[:, None]
    U8 = np.concatenate(
        [(-2.0 * data).astype(f8), col(sh), col(sl), col(sll), ones, ones, ones],
        axis=1,
    )
    V8 = np.concatenate(
        [data.astype(f8), ones, ones, ones, col(sh), col(sl), col(sll)], axis=1
    )
    return U8, V8


def _ver_in_maps(U8, V8):
    """Per-core uv buffers for the verification launch."""
    VT = np.ascontiguousarray(V8.T)  # [KV, N]
    in_maps = []
    for c in range(NCORES):
        buf = np.empty((KV, VER_COLS), VT.dtype)
        for s in range(8):
            r = 8 * s + c
            buf[:, 128 * s:128 * (s + 1)] = U8[128 * r:128 * (r + 1)].T
            b0 = 128 * (r + 1)
            w = min(VER_W, N - b0)
            a = VER_B0 + VER_W * s
            if w > 0:
                buf[:, a:a + w] = VT[:, b0:b0 + w]
            if w < VER_W:  # pad with strictly sub-diagonal (safe) columns
                buf[:, a + w:a + VER_W] = VT[:, 0:VER_W - w]
            buf[:, VER_D0 + 128 * s:VER_D0 + 128 * (s + 1)] = VT[
                :, 128 * r:128 * (r + 1)
            ]
        buf[:, VER_VS0:VER_COLS] = VT[:, 1024:N]
        in_maps.append({"uv": buf})
    return in_maps


def _ver_decision(results, rho_t, delta_t):
    """True iff the device output certifies that every point is a center."""
    import sys

    def _why(msg):
        if os.environ.get("KERNEL_DEBUG"):
            print(f"ver_decision: {msg}", file=sys.stderr)
        return False

    if not (rho_t < 1.0 - 1e-6):
        return _why(f"rho_t {rho_t} >= 1")
    t2 = max(float(delta_t), 0.0) ** 2
    if t2 > VER_T2MAX:  # count certification only covers d2 below ~theta-err
        return _why(f"t2 {t2} > {VER_T2MAX}")
    groups = _ver_groups()
    for c in range(NCORES):
        out = results[c]["vout"]  # [P, ng]
        if not np.all(np.isfinite(out)):
            return _why(f"core {c} non-finite")
        for gi, g in enumerate(groups):
            if g["eng"] == "act":
                cnt = float(out[:, gi].astype(np.float64).sum())
                expect = 1024.0 if g["diag"] else 0.0
                if abs(cnt - expect) > 0.4:
                    return _why(f"core {c} g{gi} cnt {cnt} != {expect}")
            else:
                if float(out[:, gi].min()) <= t2 + VER_MARGIN:
                    return _why(
                        f"core {c} g{gi} min {float(out[:, gi].min())}"
                    )
    return True


def _f32r(ap):
    import concourse.mybir as mybir

    return ap.bitcast(mybir.dt.float32r)



def _pe_warmup(nc, tc, inp, psum_p, mybir, n_mm=8):
    """Dense garbage-matmul burst at launch start: runs while the input DMA
    streams, trips the PE HAM un-throttle (~3.4us sustained busy) so real
    matmuls run at 2.4 GHz instead of 1.2 GHz."""
    f32 = mybir.dt.float32
    warm = inp.tile([K, P + MM_N], mybir.dt.bfloat16)
    nc.gpsimd.memset(warm[:], 1.0)
    wps = psum_p.tile([P, FD], f32, tag="psum")
    for j in range(n_mm):
        nc.tensor.matmul(
            wps[:, (j % MM_PER_G) * MM_N:((j % MM_PER_G) + 1) * MM_N],
            warm[:, :P],
            warm[:, P:P + MM_N],
            start=True,
            stop=True,
        )


def _build_l12():
    """Merged count + rho launch: dc^2 is computed ON DEVICE.

    Every core counts the SAME sample (rows 0..1023 via the shared `uvc`
    lhsT, diag-free col groups), so each core independently derives an
    identical dc^2 - no collectives. The CDF interpolation runs as tiny
    [1,8] vector ops; a PE ones-matmul does the partition reduction and a
    K=1 fp32 matmul broadcasts -1/dc^2 to all partitions for the rho phase.
    `dvec` carries host-computed control-variate corrections (in counts)
    that cancel the row/col sampling bias of the fixed sample.
    """
    import concourse.mybir as mybir
    import concourse.tile as tile
    from concourse import bacc

    f32 = mybir.dt.float32
    nc = bacc.Bacc("TRN2", debug=False, enable_asserts=False)
    bf16 = mybir.dt.bfloat16
    uv_d = nc.dram_tensor("uv", [K, ROWS + N], bf16, kind="ExternalInput")
    uvc_d = nc.dram_tensor("uvc", [K, ROWS], bf16, kind="ExternalInput")
    thr_d = nc.dram_tensor("thr", [P, NT], f32, kind="ExternalInput")
    tvec_d = nc.dram_tensor("tvec", [1, NT], f32, kind="ExternalInput")
    dvec_d = nc.dram_tensor("dvec", [1, NT], f32, kind="ExternalInput")
    cnt_d = nc.dram_tensor("counts", [P, NT], f32, kind="ExternalOutput")
    rho_d = nc.dram_tensor("rho", [P, RB], f32, kind="ExternalOutput")

    with tile.TileContext(nc) as tc:
        with (
            tc.tile_pool(name="inp", bufs=1) as inp,
            tc.tile_pool(name="stat", bufs=1) as stat,
            tc.tile_pool(name="trash", bufs=2) as trash_p,
            tc.tile_pool(name="psum", bufs=2, space="PSUM") as psum_p,
        ):
            uv_sb = inp.tile([K, ROWS + N], bf16)
            uvc_sb = inp.tile([K, ROWS], bf16)
            nc.sync.dma_start(out=uvc_sb[:], in_=uvc_d[:])
            for _g in (1, 2, 3):  # count-phase cols first; group 0 only for rho
                _a = ROWS + _g * FD
                nc.sync.dma_start(
                    out=uv_sb[:, _a:_a + FD], in_=uv_d[:, _a:_a + FD]
                )
            nc.sync.dma_start(out=uv_sb[:, 0:ROWS], in_=uv_d[:, 0:ROWS])
            nc.sync.dma_start(
                out=uv_sb[:, ROWS:ROWS + FD], in_=uv_d[:, ROWS:ROWS + FD]
            )
            thr_sb = inp.tile([P, NT], f32)
            nc.gpsimd.dma_start(out=thr_sb[:], in_=thr_d[:])
            tdv_sb = inp.tile([1, 2 * NT], f32)
            nc.gpsimd.dma_start(out=tdv_sb[:, 0:NT], in_=tvec_d[:])
            nc.gpsimd.dma_start(out=tdv_sb[:, NT:2 * NT], in_=dvec_d[:])
            cnts = stat.tile([P, NT], f32)
            warmact = stat.tile([P, 1], f32)
            nc.scalar.activation(
                warmact[:], thr_sb[:, 0:1],
                mybir.ActivationFunctionType.Sigmoid, bias=0.0, scale=1.0,
            )

            # ---- phase 1: counts over the shared sample -----------------
            for b, (m, g) in enumerate(L1_GROUPS):
                psum = psum_p.tile([P, FD], f32, tag="psum")
                for j in range(L1_W // MM_N):
                    nc.tensor.matmul(
                        psum[:, j * MM_N:(j + 1) * MM_N],
                        uvc_sb[:, m * P:(m + 1) * P],
                        uv_sb[:, ROWS + g * FD + j * MM_N: ROWS + g * FD + (j + 1) * MM_N],
                        start=True,
                        stop=True,
                    )
                t = trash_p.tile([P, L1_W], f32, tag="cntrash")
                nc.scalar.activation(
                    t[:],
                    psum[:, 0:L1_W],
                    mybir.ActivationFunctionType.Sigmoid,
                    bias=thr_sb[:, b:b + 1],
                    scale=float(-SIG_ALPHA),
                    accum_out=cnts[:, b:b + 1],
                )
            nc.gpsimd.dma_start(out=cnt_d[:], in_=cnts[:])

            # ---- phase 2: dc^2 from counts (identical on every core) ----
            ones_col = stat.tile([P, 1], f32)
            nc.vector.memset(ones_col[:], 1.0)
            ps_tot = psum_p.tile([1, NT], f32, tag="psum")
            nc.tensor.matmul(ps_tot[:], ones_col[:], cnts[:], start=True, stop=True)
            w = stat.tile([1, 8 * NT], f32)  # scratch lanes along free dim
            q = w[:, 0:NT]
            nc.vector.tensor_tensor(
                out=q, in0=ps_tot[:], in1=tdv_sb[:, NT:2 * NT],
                op=mybir.AluOpType.subtract,
            )
            NB_ = NT - 1
            a_ = w[:, NT:NT + NB_]
            nc.vector.tensor_scalar(
                out=a_, in0=q[:, 0:NB_], scalar1=CSTAR, scalar2=None,
                op0=mybir.AluOpType.is_le,
            )
            b_ = w[:, 2 * NT:2 * NT + NB_]
            nc.vector.tensor_scalar(
                out=b_, in0=q[:, 1:NT], scalar1=CSTAR, scalar2=None,
                op0=mybir.AluOpType.is_gt,
            )
            sel = w[:, 3 * NT:3 * NT + NB_]
            nc.vector.tensor_tensor(out=sel, in0=a_, in1=b_, op=mybir.AluOpType.mult)
            den = w[:, 4 * NT:4 * NT + NB_]
            nc.vector.tensor_tensor(
                out=den, in0=q[:, 1:NT], in1=q[:, 0:NB_],
                op=mybir.AluOpType.subtract,
            )
            rec = w[:, 5 * NT:5 * NT + NB_]
            nc.vector.reciprocal(rec, den)
            num = w[:, 6 * NT:6 * NT + NB_]
            nc.vector.tensor_scalar(
                out=num, in0=q[:, 0:NB_], scalar1=-1.0, scalar2=CSTAR,
                op0=mybir.AluOpType.mult, op1=mybir.AluOpType.add,
            )
            fr = w[:, 7 * NT:7 * NT + NB_]
            nc.vector.tensor_tensor(out=fr, in0=num, in1=rec, op=mybir.AluOpType.mult)
            nc.vector.tensor_scalar(
                out=fr, in0=fr, scalar1=float(DC2_STEP), scalar2=None,
                op0=mybir.AluOpType.mult,
            )
            nc.vector.tensor_tensor(
                out=fr, in0=fr, in1=tdv_sb[:, 0:NB_], op=mybir.AluOpType.add
            )
            nc.vector.tensor_tensor(out=fr, in0=fr, in1=sel, op=mybir.AluOpType.mult)
            sc = stat.tile([1, 4], f32)
            nc.vector.tensor_reduce(
                sc[:, 0:1], fr[:], axis=mybir.AxisListType.X, op=mybir.AluOpType.add
            )
            nc.vector.tensor_reduce(
                sc[:, 1:2], sel[:], axis=mybir.AxisListType.X, op=mybir.AluOpType.add
            )
            # guard: if no bracket, fall back to the grid center
            nc.vector.tensor_scalar(
                out=sc[:, 2:3], in0=sc[:, 1:2], scalar1=float(-DC2_CENTER),
                scalar2=float(DC2_CENTER), op0=mybir.AluOpType.mult,
                op1=mybir.AluOpType.add,
            )
            nc.vector.tensor_tensor(
                out=sc[:, 0:1], in0=sc[:, 0:1], in1=sc[:, 2:3],
                op=mybir.AluOpType.add,
            )
            nc.vector.reciprocal(sc[:, 3:4], sc[:, 0:1])
            nc.vector.tensor_scalar(
                out=sc[:, 3:4], in0=sc[:, 3:4], scalar1=-1.0, scalar2=None,
                op0=mybir.AluOpType.mult,
            )
            ones_row = stat.tile([1, P], f32)
            nc.vector.memset(ones_row[:], 1.0)
            ps_b = psum_p.tile([P, 1], f32, tag="psum")
            nc.tensor.matmul(ps_b[:], ones_row[:], sc[:, 3:4], start=True, stop=True)
            scl_sb = stat.tile([P, 1], f32)
            nc.vector.tensor_copy(scl_sb[:], ps_b[:])

            # ---- phase 3: rho ------------------------------------------
            parts = stat.tile([P, RB * G], f32)
            rho_sb = stat.tile([P, RB], f32)
            for m in range(RB):
                for g in range(G):
                    psum = psum_p.tile([P, FD], f32, tag="psum")
                    for j in range(MM_PER_G):
                        nc.tensor.matmul(
                            psum[:, j * MM_N:(j + 1) * MM_N],
                            uv_sb[:, m * P:(m + 1) * P],
                            uv_sb[:, ROWS + g * FD + j * MM_N: ROWS + g * FD + (j + 1) * MM_N],
                            start=True,
                            stop=True,
                        )
                    t = trash_p.tile([P, FD], f32, tag="trash")
                    q2 = m * G + g
                    nc.scalar.activation(
                        t[:],
                        psum[:],
                        mybir.ActivationFunctionType.Exp,
                        bias=0.0,
                        scale=scl_sb[:, 0:1],
                        accum_out=parts[:, q2:q2 + 1],
                    )
                nc.vector.tensor_reduce(
                    rho_sb[:, m:m + 1],
                    parts[:, m * G:(m + 1) * G],
                    axis=mybir.AxisListType.X,
                    op=mybir.AluOpType.add,
                )
            nc.sync.dma_start(out=rho_d[:], in_=rho_sb[:])
    nc.compile()
    return nc


def _build_l3():
    """Delta pass on rho-sorted data (round-robin block interleaving).

    Core c holds sorted row-blocks b = 8m + c (m = 0..7). For local block m:
      boundary col-group g_b = m//2, window base w_lo = 1024*(m%2)
      (cutoffs of every core's block-m rows lie in [w_lo, w_lo+1024) of
      group g_b, ties aside - those are patched on host).
    Structure per block:
      groups g < g_b:                plain min-reduce of the whole group
      boundary prefix [0, w_lo):     plain min-reduce (odd m only)
      boundary window [w_lo,+1024):  penalty mask (iota >= cutrel)*BIG, add,
                                     min-reduce
      columns beyond w_lo+1024 and groups g > g_b: skipped entirely.
    """
    import concourse.mybir as mybir
    import concourse.tile as tile
    from concourse import bacc

    f32 = mybir.dt.float32
    nc = bacc.Bacc("TRN2", debug=False, enable_asserts=False)
    bf16 = mybir.dt.bfloat16
    uv_d = nc.dram_tensor("uv", [K, ROWS + N], bf16, kind="ExternalInput")
    cut_d = nc.dram_tensor("cut", [P, RB], f32, kind="ExternalInput")
    iota_d = nc.dram_tensor("iota", [P, WW], f32, kind="ExternalInput")
    dmin_d = nc.dram_tensor("dmin", [P, RB * NCOL], f32, kind="ExternalOutput")

    with tile.TileContext(nc) as tc:
        with (
            tc.tile_pool(name="inp", bufs=1) as inp,
            tc.tile_pool(name="stat", bufs=1) as stat,
            tc.tile_pool(name="trash", bufs=3) as trash_p,
            tc.tile_pool(name="pen", bufs=3) as pen_p,
            tc.tile_pool(name="psum", bufs=2, space="PSUM") as psum_p,
        ):
            uv_sb = inp.tile([K, ROWS + N], bf16)
            nc.sync.dma_start(out=uv_sb[:, 0:ROWS], in_=uv_d[:, 0:ROWS])
            for _g in range(G):
                _a = ROWS + _g * FD
                nc.sync.dma_start(
                    out=uv_sb[:, _a:_a + FD], in_=uv_d[:, _a:_a + FD]
                )
            cut_sb = inp.tile([P, RB], f32)
            nc.gpsimd.dma_start(out=cut_sb[:], in_=cut_d[:])
            iota_sb = inp.tile([P, WW], f32)
            nc.gpsimd.dma_start(out=iota_sb[:], in_=iota_d[:])
            dmin_sb = stat.tile([P, RB * NCOL], f32)

            for m in range(RB):
                gb = m // 2
                w_lo = WW * (m % 2)
                pen = pen_p.tile([P, WW], f32, tag="pen")
                # cutrel (host-clamped to [0, WW]) is relative to w_lo
                nc.vector.tensor_scalar(
                    out=pen[:],
                    in0=iota_sb[:],
                    scalar1=cut_sb[:, m:m + 1],
                    scalar2=PEN_BIG,
                    op0=mybir.AluOpType.is_ge,
                    op1=mybir.AluOpType.mult,
                )
                for g in range(gb + 1):
                    ncols = FD if g < gb else w_lo + WW
                    psum = psum_p.tile([P, FD], f32, tag="psum")
                    for j in range(ncols // MM_N):
                        nc.tensor.matmul(
                            psum[:, j * MM_N:(j + 1) * MM_N],
                            uv_sb[:, m * P:(m + 1) * P],
                            uv_sb[:, ROWS + g * FD + j * MM_N: ROWS + g * FD + (j + 1) * MM_N],
                            start=True,
                            stop=True,
                        )
                    q = m * NCOL + g
                    if g < gb:
                        nc.vector.tensor_reduce(
                            dmin_sb[:, q:q + 1],
                            psum[:],
                            axis=mybir.AxisListType.X,
                            op=mybir.AluOpType.min,
                        )
                    else:
                        if w_lo > 0:
                            nc.vector.tensor_reduce(
                                dmin_sb[:, q:q + 1],
                                psum[:, 0:w_lo],
                                axis=mybir.AxisListType.X,
                                op=mybir.AluOpType.min,
                            )
                        t = trash_p.tile([P, WW], f32, tag="trash")
                        nc.vector.tensor_tensor(
                            out=t[:],
                            in0=psum[:, w_lo:w_lo + WW],
                            in1=pen[:],
                            op=mybir.AluOpType.add,
                        )
                        nc.vector.tensor_reduce(
                            dmin_sb[:, m * NCOL + G:m * NCOL + G + 1],
                            t[:],
                            axis=mybir.AxisListType.X,
                            op=mybir.AluOpType.min,
                        )
            nc.gpsimd.dma_start(out=dmin_d[:], in_=dmin_sb[:])
    nc.compile()
    return nc


_BUILDERS = {"l12": _build_l12, "l3": _build_l3, "ver": _build_ver}


def _get_program(name):
    if name not in _programs:
        _programs[name] = _BUILDERS[name]()
    return _programs[name]


TIMINGS = []  # (name, exec_time_ns) per launch, appended by _run


def _run(name, in_maps, trace=None):
    from concourse.bass_utils import run_bass_kernel_spmd

    if trace is None:
        trace = bool(int(os.environ.get("KERNEL_TRACE", "0")))
    nc = _get_program(name)
    res = run_bass_kernel_spmd(
        nc, in_maps, core_ids=list(range(NCORES)), trace=trace
    )
    TIMINGS.append((name, res.exec_time_ns))
    return res


def _augmented(data):
    """U (lhs rows) and V (rhs cols) of the K=68 augmented distance GEMM.

    bf16 operands with sq split into a bf16 hi+lo pair: d2 error ~0.04 abs
    (~5e-4 relative at the dc^2 scale), far inside every decision margin.
    """
    import ml_dtypes

    bf = ml_dtypes.bfloat16
    sq = np.einsum("ij,ij->i", data, data, dtype=np.float32).astype(np.float32)
    sqh = sq.astype(bf)
    sql = (sq - sqh.astype(np.float32)).astype(bf)
    ones = np.ones((N, 1), bf)
    zcol = lambda a: a[:, None]
    U = np.concatenate(
        [(-2.0 * data).astype(bf), zcol(sqh), zcol(sql), ones, ones], axis=1
    )
    V = np.concatenate(
        [data.astype(bf), ones, ones, zcol(sqh), zcol(sql)], axis=1
    )
    return U, V, sq


def _erf(x):
    """Abramowitz-Stegun 7.1.26 vectorized erf (|err| < 1.5e-7)."""
    s = np.sign(x)
    x = np.abs(x)
    t = 1.0 / (1.0 + 0.3275911 * x)
    y = 1.0 - (
        ((((1.061405429 * t - 1.453152027) * t) + 1.421413741) * t - 0.284496736)
        * t
        + 0.254829592
    ) * t * np.exp(-x * x)
    return s * y


def _phi(z):
    return 0.5 * (1.0 + _erf(z / np.sqrt(2.0)))


NGRID = 256


def _cv_corrections(sq):
    """Control-variate count corrections for the fixed count sample.

    Model P(d2 < t | sq_i, sq_j) ~ Phi((t - sq_i - sq_j)/(2 sqrt(sq_i sq_j/D)))
    and subtract the predicted row/col selection bias of the sampled
    rows/cols relative to the full point set.
    """
    sq64 = sq.astype(np.float64)
    step = N // NGRID
    grid = np.sort(sq64)[step // 2::step][:NGRID]

    def h(t, svals):
        s = svals[:, None]
        sp = grid[None, :]
        z = (t - s - sp) / (2.0 * np.sqrt(np.maximum(s * sp, 1e-9) / D))
        return _phi(z).mean(axis=1)

    dvec = np.zeros(NT)
    for b, (m, g) in enumerate(L1_GROUPS):
        t = float(DC2_GRID[b])
        h_all = h(t, grid).mean()
        d_row = h(t, sq64[m * P:(m + 1) * P]).mean() - h_all
        d_col = h(t, sq64[g * FD:g * FD + L1_W]).mean() - h_all
        dvec[b] = (d_row + d_col) * (P * L1_W)
    return dvec.astype(np.float32).reshape(1, NT)


def _interp_dc2(counts_by_core):
    """counts_by_core: list of [P, NT] arrays -> dc^2 via CDF interpolation."""
    M = float(N) * float(N)
    k_pos = PCT / 100.0 * (M - 1.0)
    p_off = (k_pos - N) / (M - N)  # diag cells (d2=0) all fall below any t_b

    tot = np.zeros(NT, np.float64)
    denom = np.zeros(NT, np.float64)
    for c in range(NCORES):
        cc = counts_by_core[c].astype(np.float64).sum(axis=0)  # [NT]
        for b, (m, g) in enumerate(L1_GROUPS):
            row0 = c * ROWS + m * P
            off = row0 - g * FD
            has_diag = 0 <= off <= L1_W - P
            tot[b] += cc[b] - (P if has_diag else 0)
            denom[b] += P * L1_W - (P if has_diag else 0)
    p_hat = tot / denom
    # p_hat should be increasing in b; enforce monotonicity for safety
    p_mono = np.maximum.accumulate(p_hat)
    if not (p_mono[0] <= p_off <= p_mono[-1]):
        return None  # bracket miss -> caller falls back to exact host path
    b_hi = int(np.searchsorted(p_mono, p_off, side="left"))
    if b_hi == 0:
        return float(DC2_GRID[0])
    b_lo = b_hi - 1
    p_lo, p_hi_v = p_mono[b_lo], p_mono[b_hi]
    frac = 0.0 if p_hi_v <= p_lo else (p_off - p_lo) / (p_hi_v - p_lo)
    return float(DC2_GRID[b_lo] + frac * (DC2_GRID[b_hi] - DC2_GRID[b_lo]))


def _host_fallback(data, rho_t, delta_t):
    """Pure-numpy reference path (only used if device assumptions break)."""
    data = np.asarray(data, np.float32)
    sq = np.sum(data * data, axis=1)
    d2 = sq[:, None] + sq[None, :] - 2.0 * (data @ data.T)
    dist = np.sqrt(np.maximum(d2, 0.0), dtype=np.float32)
    dc = np.percentile(dist, PCT)
    rho = np.exp(-((dist / dc) ** 2)).sum(axis=1).astype(np.float32)
    higher = rho[None, :] > rho[:, None]
    masked = np.where(higher, dist, np.inf)
    delta_m = masked.min(axis=1)
    nhd_m = masked.argmin(axis=1)
    has = higher.any(axis=1)
    delta = np.where(has, delta_m, dist.max(axis=1))
    nhd = np.where(has, nhd_m, np.arange(N))
    return _finish_labels(rho, delta, nhd, rho_t, delta_t)


def _finish_labels(rho, delta, nhd, rho_t, delta_t):
    is_center = (rho > rho_t) & (delta > delta_t)
    center_rank = np.cumsum(is_center.astype(np.int32)) - 1
    labels = np.where(is_center, center_rank, -1).astype(np.int32)
    order = np.argsort(-rho, kind="stable")
    for i in order:
        if labels[i] < 0:
            labels[i] = labels[nhd[i]]
    return labels


def kernel(data, rho_threshold, delta_threshold):
    data = np.ascontiguousarray(np.asarray(data, dtype=np.float32))
    assert data.shape == (N, D)
    rho_t = float(np.asarray(rho_threshold))
    delta_t = float(np.asarray(delta_threshold))

    # ---- fast path: certify "every point is a center" ------------------
    U8, V8 = _augmented_f8(data)
    rver = _run("ver", _ver_in_maps(U8, V8))
    if _ver_decision(rver.results, rho_t, delta_t):
        return np.arange(N, dtype=np.int32)

    U, V, sq = _augmented(data)
    VT = V.T  # [K, N]

    # ---- L12: counts -> on-device dc^2 -> rho (single launch) ----------
    thr = np.broadcast_to(
        (SIG_ALPHA * DC2_GRID).astype(np.float32)[None, :], (P, NT)
    ).copy()
    tvec = DC2_GRID.astype(np.float32).reshape(1, NT)
    dvec = _cv_corrections(sq)
    uvc = np.ascontiguousarray(np.concatenate([U[0:ROWS].T, VT], axis=1)[:, 0:ROWS])
    in_maps = [
        {
            "uv": np.ascontiguousarray(
                np.concatenate([U[c * ROWS:(c + 1) * ROWS].T, VT], axis=1)
            ),
            "uvc": uvc,
            "thr": thr,
            "tvec": tvec,
            "dvec": dvec,
        }
        for c in range(NCORES)
    ]
    r12 = _run("l12", in_maps)

    # validate the on-device dc interpolation from the counts output
    q = r12.results[0]["counts"].astype(np.float64).sum(axis=0) - dvec[0].astype(
        np.float64
    )
    brackets = [
        b for b in range(NT - 1) if q[b] <= CSTAR < q[b + 1]
    ]
    if len(brackets) != 1 or not np.all(np.diff(q) > 0):
        return _host_fallback(data, rho_t, delta_t)

    rho = np.empty(N, np.float32)
    for c in range(NCORES):
        out = r12.results[c]["rho"]  # [P, RB]
        rho[c * ROWS:(c + 1) * ROWS] = out.T.reshape(-1)
    if not np.all(np.isfinite(rho)) or rho.min() < 0.5 or rho.max() > N + 1:
        return _host_fallback(data, rho_t, delta_t)

    # ---- host: sort by rho desc; prefix cutoffs ------------------------
    order = np.argsort(-rho, kind="stable")
    rho_sorted = rho[order]
    # c_i = #points with rho strictly greater (ties excluded)
    cuts = np.searchsorted(-rho_sorted, -rho_sorted, side="left").astype(np.int64)

    data_p = data[order]
    sq_p = sq[order]
    Up = U[order]
    Vp = V[order]
    rhs_p = np.ascontiguousarray(Vp.T)

    # round-robin block interleave: core c <- sorted blocks 8m + c
    NB = N // P  # 64 sorted row-blocks
    blk_rows = np.arange(N).reshape(NB, P)
    core_rows = [blk_rows[np.arange(RB) * NCORES + c].reshape(-1) for c in range(NCORES)]

    iota_in = np.broadcast_to(
        np.arange(WW, dtype=np.float32)[None, :], (P, WW)
    ).copy()
    in_maps = []
    for c in range(NCORES):
        rows = core_rows[c]
        cutrel = np.empty((P, RB), np.float32)
        for m in range(RB):
            base = (m // 2) * FD + WW * (m % 2)
            cutrel[:, m] = np.clip(cuts[rows[m * P:(m + 1) * P]] - base, 0, WW)
        in_maps.append(
            {
                "uv": np.ascontiguousarray(
                    np.concatenate([Up[rows].T, rhs_p], axis=1)
                ),
                "cut": cutrel,
                "iota": iota_in,
            }
        )
    r3 = _run("l3", in_maps)
    # dmin[i] holds per-source minima; dcol[k] = (col_base, col_len) of source k
    dmin = np.full((N, NCOL), np.inf, np.float32)
    for c in range(NCORES):
        out = r3.results[c]["dmin"]  # [P, RB*NCOL]
        rows = core_rows[c]
        for m in range(RB):
            gb = m // 2
            w_lo = WW * (m % 2)
            blk = rows[m * P:(m + 1) * P]
            for g in range(gb):
                dmin[blk, g] = out[:, m * NCOL + g]
            if w_lo > 0:
                dmin[blk, gb] = out[:, m * NCOL + gb]
            dmin[blk, G] = out[:, m * NCOL + G]

    # ---- host: delta, fallback rows, centers, nhd (lazy), labels -------
    delta2_sorted = dmin.min(axis=1)

    # rho-tie rows whose cutoff dips below their block's boundary group: the
    # device's full-group reduce included a few extra columns; fix exactly.
    win_base = ((np.arange(N) // P) // NCORES) * WW  # 1024*m per sorted row
    straddle_fix = {}
    for i in np.nonzero(cuts < win_base)[0]:
        cut = int(cuts[i])
        if cut == 0:
            delta2_sorted[i] = np.inf
            continue
        d2row = sq_p[i] + sq_p[:cut] - 2.0 * (data_p[:cut] @ data_p[i])
        j = int(np.argmin(d2row))
        delta2_sorted[i] = d2row[j]
        straddle_fix[i] = j

    empty = delta2_sorted >= EMPTY_SENTINEL  # no higher-density point
    delta_sorted = np.sqrt(np.maximum(delta2_sorted, 0.0), dtype=np.float32)
    for i in np.nonzero(empty)[0]:
        d2row = sq_p[i] + sq_p - 2.0 * (data_p @ data_p[i])
        delta_sorted[i] = np.sqrt(max(float(np.max(np.maximum(d2row, 0.0))), 0.0))

    delta = np.empty(N, np.float32)
    delta[order] = delta_sorted

    is_center = (rho > rho_t) & (delta > delta_t)
    center_rank = np.cumsum(is_center.astype(np.int32)) - 1
    labels = np.where(is_center, center_rank, -1).astype(np.int32)

    need_nhd = ~is_center[order]  # sorted positions whose label must propagate
    nhd = np.arange(N, dtype=np.int64)  # default: self (matches reference)
    for i in np.nonzero(need_nhd)[0]:
        if empty[i]:
            continue  # nhd stays self, as in reference
        if i in straddle_fix:
            nhd[order[i]] = order[straddle_fix[i]]
            continue
        k = int(np.argmin(dmin[i]))
        m = (i // P) // NCORES
        gb = m // 2
        w_lo = WW * (m % 2)
        if k == G:
            c0, clen = gb * FD + w_lo, WW
        elif k == gb:
            c0, clen = gb * FD, w_lo
        else:
            c0, clen = k * FD, FD
        end_local = int(np.clip(cuts[i] - c0, 0, clen))
        cols = slice(c0, c0 + end_local)
        d2part = sq_p[i] + sq_p[cols] - 2.0 * (data_p[cols] @ data_p[i])
        j_local = int(np.argmin(d2part))
        nhd[order[i]] = order[c0 + j_local]

    for i in order:
        if labels[i] < 0:
            labels[i] = labels[nhd[i]]
    return labels.astype(np.int32)

